# revision 56
# baseline (speedup 1.0000x reference)
"""Trainium2 Bass kernel for nn_Block_77318001263203 (dense transformer block).

Distribution over 8 NeuronCores: data-parallel over batch (2 groups of 4
cores) x tensor-parallel over heads (4 heads/core) for attention+proj,
4-way-chunked bf16 ReduceScatter of the proj partials over each 4-core
group (each chunk hands every rank one 128-token block, so rank r owns
the strided token set {512*ck + 128*r + j}), then token-parallel FFN with
replicated FFN weights — no second collective.

v3 schedule (vs v2): built for PE clock density — the HAM activity
monitor halves the PE clock whenever its duty cycle sags, and v2 spent
~40% of its span at 1.2 GHz.
  - PE warm-up: a burst of identity transposes runs during the initial
    weight DMAs so the first real matmuls start at 2.4 GHz.
  - Startup DMAs spread over four rings (sync/scalar/vector/gpsimd).
  - QKV token tiles and attention row blocks interleave (tt0, tt1, qt0,
    tt2, qt1, tt3, qt2, qt3) so attention's scalar-engine exp stream
    starts ~40us earlier and overlaps QKV matmuls.
  - Attention processes key blocks in 2-block waves with one fused exp
    per wave (N=1024 activation ops instead of N=512) and the AV matmuls
    trailing one wave for latency hiding.
  - FFN1 runs token tiles {0,1} as one N=256 pass (vector-relu), tiles
    2,3 as N=128 passes (scalar-relu); FFN2 streams w2 row-chunks once
    per token-tile pair with hT stationary and N=512 moving, accumulating
    both tiles' [128,1024] outputs per chunk.
  - Phase B is sequential after the last proj chunk: the ReduceScatter
    latency hides behind the {0,1}-pair FFN work and the tail after RS3
    lands is just tile 3's ln1 -> ffn1 -> ffn2 chain.

All matmuls bf16 (fp32 PSUM accumulate); b_proj is folded into the xs
residual host-side. Known HW faults respected from v2: no bf16 matmul
operands at partition offset 64, no f32r tile_position packing.

kernel(**inputs) takes the FULL inputs from setup_inputs() and returns the
FULL [2, 2048, 1024] float32 output.
"""

import numpy as np
import ml_dtypes

import concourse.bass as bass
import concourse.mybir as mybir
import concourse.tile as tile
from concourse import bacc
from concourse.bass_utils import run_bass_kernel_spmd
from concourse.masks import make_identity

# problem dims (hardcoded per the harness contract)
B, S, D = 2, 2048, 1024
H, HS, F = 16, 64, 4096
EPS = 1e-5
P = 128
NCORES = 8
TP = 4  # cores per batch group
HPC = H // TP  # heads per core = 4
SL = S // TP  # tokens owned per core = 512 (4 strided blocks of 128)
QT = 512  # query row tile (attention row granularity)
SUB = 256  # score/exp subtile width
KB = 128  # key block
NCK = 4  # reduce-scatter chunks
NEG = -1.0e9  # additive causal mask (exp underflows to exactly 0)

f32 = mybir.dt.float32
bf16 = mybir.dt.bfloat16
bfnp = ml_dtypes.bfloat16

REPLICA_GROUPS = [[0, 1, 2, 3], [4, 5, 6, 7]]


def _bcast_row_ap(t, row, width):
    """DMA-source AP broadcasting row `row` of DRAM tensor t to 128 partitions."""
    return bass.AP(tensor=t, offset=row * width, ap=[[0, P], [1, width]])


def build_bass():
    import os

    # debug bisection: 1=QKV, 2=+attention rows, 3=+proj/RS, 4=full
    STAGE = int(os.environ.get("KSTAGE", "4"))
    nc = bacc.Bacc("TRN2", target_bir_lowering=False, debug=False, num_devices=NCORES)

    xT = nc.dram_tensor("xT", [D, S], bf16, kind="ExternalInput").ap()
    xs = nc.dram_tensor("xs", [SL, D], f32, kind="ExternalInput").ap()
    wq2 = nc.dram_tensor("wq2", [D, HPC * HS], bf16, kind="ExternalInput").ap()
    wk2 = nc.dram_tensor("wk2", [D, HPC * HS], bf16, kind="ExternalInput").ap()
    wv4 = nc.dram_tensor("wv4", [D, HPC * HS], bf16, kind="ExternalInput").ap()
    wp = nc.dram_tensor("wp", [HPC * HS, D], bf16, kind="ExternalInput").ap()
    w1 = nc.dram_tensor("w1", [D, F], bf16, kind="ExternalInput").ap()
    w2 = nc.dram_tensor("w2", [F, D], bf16, kind="ExternalInput").ap()
    cvec = nc.dram_tensor("cvec", [6, D], f32, kind="ExternalInput").ap()
    b1d = nc.dram_tensor("b1d", [F], f32, kind="ExternalInput").ap()
    out = nc.dram_tensor("out", [SL, D], f32, kind="ExternalOutput").ap()

    # per-chunk collective bounce buffers (separate tensors -> precise deps)
    rs_in = [nc.dram_tensor(f"rs_in{c}", [QT, D], bf16) for c in range(NCK)]
    rs_out = [nc.dram_tensor(f"rs_out{c}", [P, D], bf16) for c in range(NCK)]
    # tiny dummy collective fired at kernel start: absorbs the CC stream's
    # first-trigger startup latency (~11us) so RS chunk 0 starts promptly
    cc_warm_in = nc.dram_tensor("cc_warm_in", [TP, 1], bf16)
    cc_warm_out = nc.dram_tensor("cc_warm_out", [1, 1], bf16)

    # additive causal mask [all-NEG block | lower-triangle-NEG block]:
    # mfull[:, KB:] is the triangle alone (keep 0 where key t <= query q).
    tri = np.where(
        np.arange(KB)[:, None] <= np.arange(KB)[None, :], 0.0, NEG
    ).astype(np.float32)
    full = np.concatenate([np.full((KB, KB), NEG, np.float32), tri], axis=1)
    m_full_dram = nc.inline_tensor(np.ascontiguousarray(full), name="mask_full")

    with tile.TileContext(nc) as tc:
        with tc.tile_pool(name="const", bufs=1) as constp:
            identb = constp.tile([P, P], bf16)
            make_identity(nc, identb)
            eps_t = constp.tile([P, 1], f32)
            nc.vector.memset(eps_t, EPS)
            # const tiles allocated here; their broadcast DMAs (which write
            # 128x the source bytes into SBUF) are deferred until after the
            # startup-critical QKV loads are in the rings
            b1_sb = constp.tile([P, F // P], f32)
            g1b = constp.tile([P, D], f32)
            be1b = constp.tile([P, D], f32)
            g2b = constp.tile([P, D], f32)
            be2b = constp.tile([P, D], f32)
            b2b = constp.tile([P, D], bf16)

            keep_cm = tc.tile_pool(name="keep", bufs=1)
            keep = keep_cm.__enter__()
            mfull_sb = keep.tile([P, 2 * KB], f32, tag="mfull")
            mtri_sb = mfull_sb[:, KB : 2 * KB]

            # wp's dma is emitted after wq below (same ring, wq is urgent)
            wp_sb = keep.tile([P, (HPC * HS) // P, D], bf16, tag="wp")

            # attention working set. q/k live on partitions 0-63 with the
            # head parity in the free dim: bf16 matmul operands fault on
            # HW at partition offset 64, so every score matmul reads
            # partition-offset-0 slices.
            q2T = keep.tile([HS, 2, 2, S], bf16, tag="q2T")
            k2T = keep.tile([HS, 2, 2, S], bf16, tag="k2T")
            v4e = keep.tile([P, S // P, HPC * (HS + 1)], bf16, tag="v4e")
            ones4 = keep.tile([P, HPC, 1], bf16, tag="ones4")
            nc.vector.memset(ones4, 1.0)

            # phase B persistents
            w1_sb = keep.tile([P, D // P, F], bf16, tag="w1")
            x1T = keep.tile([P, D // P, SL], bf16, tag="x1T")
            x1r = keep.tile([P, SL // P, D], bf16, tag="x1r")
            hT = keep.tile([P, F // P, SL // 2], bf16, tag="hT")

            # PE warm-up: ~48 back-to-back transposes (~5us of PE activity)
            # run during the initial weight DMAs so the HAM clock gate is
            # released before the first real matmul.
            with tc.tile_pool(name="ps_warm", bufs=1, space="PSUM") as pswarm:
                wtile = pswarm.tile([P, P], bf16, tag="warm")
                for _ in range(48):
                    nc.tensor.transpose(wtile, identb, identb)
            if STAGE >= 3:
                nc.gpsimd.collective_compute(
                    "ReduceScatter",
                    mybir.AluOpType.add,
                    replica_groups=REPLICA_GROUPS,
                    ins=[cc_warm_in.ap().opt()],
                    outs=[cc_warm_out.ap().opt()],
                )

            # ---------------- interleaved QKV + attention ----------------
            # SBUF pool stack: pools that outlive QKV open first so the
            # QKV pools (wqkv, xr) can close mid-kernel in LIFO order.
            atp_cm = tc.tile_pool(name="atp", bufs=1)
            atp = atp_cm.__enter__()
            smallp_cm = tc.tile_pool(name="smallp", bufs=2)
            smallp = smallp_cm.__enter__()
            normp_cm = tc.tile_pool(name="normp", bufs=2)
            normp = normp_cm.__enter__()
            projp_cm = tc.tile_pool(name="projp", bufs=1)
            projp = projp_cm.__enter__()
            zp_cm = tc.tile_pool(name="zp", bufs=2)
            zp = zp_cm.__enter__()
            ln1p_cm = tc.tile_pool(name="ln1p", bufs=1)
            ln1p = ln1p_cm.__enter__()

            # QKV weights in ko-half loads so the first accumulation chunks
            # can begin before the full weight tile lands
            wqkv_cm = tc.tile_pool(name="wqkv", bufs=1)
            wqkvp = wqkv_cm.__enter__()
            wq_sb = wqkvp.tile([P, D // P, HPC * HS], bf16, tag="wq")
            wk_sb = wqkvp.tile([P, D // P, HPC * HS], bf16, tag="wk")
            wv_sb = wqkvp.tile([P, D // P, HPC * HS], bf16, tag="wv")
            for h in range(4):
                hsl = slice(h * 2, (h + 1) * 2)
                rsl = slice(h * (D // 4), (h + 1) * (D // 4))
                nc.scalar.dma_start(
                    wq_sb[:, hsl, :],
                    wq2[rsl, :].rearrange("(ko p) m -> p ko m", p=P),
                )
                nc.gpsimd.dma_start(
                    wk_sb[:, hsl, :],
                    wk2[rsl, :].rearrange("(ko p) m -> p ko m", p=P),
                )
                nc.gpsimd.dma_start(
                    wv_sb[:, hsl, :],
                    wv4[rsl, :].rearrange("(ko p) m -> p ko m", p=P),
                )
            nc.scalar.dma_start(wp_sb, wp.rearrange("(ko p) n -> p ko n", p=P))

            def deferred_consts():
                """Const loads emitted behind the urgent QKV ring traffic."""
                nc.sync.dma_start(mfull_sb, m_full_dram.ap())
                nc.sync.dma_start(b1_sb, b1d.rearrange("(ko p) -> p ko", p=P))
                nc.sync.dma_start(g1b, _bcast_row_ap(cvec.tensor, 1, D))
                nc.sync.dma_start(be1b, _bcast_row_ap(cvec.tensor, 2, D))
                nc.sync.dma_start(g2b, _bcast_row_ap(cvec.tensor, 3, D))
                nc.sync.dma_start(be2b, _bcast_row_ap(cvec.tensor, 4, D))

            xr_cm = tc.tile_pool(name="xrp", bufs=1)
            xrp = xr_cm.__enter__()
            big_cm = tc.tile_pool(name="ps_big", bufs=2, space="PSUM")
            psbig = big_cm.__enter__()
            sc_cm = tc.tile_pool(name="ps_sc", bufs=2, space="PSUM")
            pssc = sc_cm.__enter__()
            av_cm = tc.tile_pool(name="ps_av", bufs=2, space="PSUM")
            psav = av_cm.__enter__()

            def qkv_tile(tt):
                xr = xrp.tile([P, D // P, QT], bf16, tag="xr", name=f"xr_{tt}")
                # quarter-loads so the first ko chunks' matmuls can start
                # before the whole tile has landed
                for h in range(4):
                    nc.sync.dma_start(
                        xr[:, h * 2 : (h + 1) * 2, :],
                        xT[h * (D // 4) : (h + 1) * (D // 4),
                           tt * QT : (tt + 1) * QT].rearrange(
                            "(ko p) m -> p ko m", p=P
                        ),
                    )
                for hp in range(2):
                    qps = psbig.tile([P, QT], f32, tag="big", name=f"qps_{tt}_{hp}")
                    for ko in range(D // P):
                        nc.tensor.matmul(
                            qps,
                            wq_sb[:, ko, hp * P : (hp + 1) * P],
                            xr[:, ko, :],
                            start=(ko == 0),
                            stop=(ko == D // P - 1),
                        )
                    for par in range(2):
                        nc.vector.tensor_copy(
                            q2T[:, par, hp, tt * QT : (tt + 1) * QT],
                            qps[par * HS : (par + 1) * HS, :],
                        )
                    kps = psbig.tile([P, QT], f32, tag="big", name=f"kps_{tt}_{hp}")
                    for ko in range(D // P):
                        nc.tensor.matmul(
                            kps,
                            wk_sb[:, ko, hp * P : (hp + 1) * P],
                            xr[:, ko, :],
                            start=(ko == 0),
                            stop=(ko == D // P - 1),
                        )
                    for par in range(2):
                        nc.vector.tensor_copy(
                            k2T[:, par, hp, tt * QT : (tt + 1) * QT],
                            kps[par * HS : (par + 1) * HS, :],
                        )
                for mt in range(QT // P):
                    vps = psbig.tile(
                        [P, HPC * HS], f32, tag="big",
                        padded_shape=[P, QT], name=f"vps_{tt}_{mt}",
                    )
                    for ko in range(D // P):
                        nc.tensor.matmul(
                            vps,
                            xr[:, ko, mt * P : (mt + 1) * P],
                            wv_sb[:, ko, :],
                            start=(ko == 0),
                            stop=(ko == D // P - 1),
                        )
                    idx = tt * (QT // P) + mt
                    vv = v4e[:, idx, :].rearrange("p (h e) -> p h e", e=HS + 1)
                    nc.vector.tensor_copy(
                        vv[:, :, 0:HS],
                        vps.rearrange("p (h e) -> p h e", e=HS),
                    )
                    nc.vector.tensor_copy(vv[:, :, HS : HS + 1], ones4)
                # the big w1 load rides behind the latency-critical QKV
                # loads in 2MB chunks (needed only from attention(3) on)
                nc.scalar.dma_start(
                    w1_sb[:, :, tt * (F // 4) : (tt + 1) * (F // 4)],
                    w1[:, tt * (F // 4) : (tt + 1) * (F // 4)].rearrange(
                        "(ko p) m -> p ko m", p=P
                    ),
                )

            # PE filler work items (thunks) spliced between attention waves:
            # keeps the PE duty cycle high through the scalar-bound exp
            # stream so the HAM clock gate stays released, and moves phase-B
            # matmuls off the critical tail.
            fillers = []

            def attention(qt, fill_from=10 ** 9):
                """Score/exp/AV waves + normalize for query rows
                [qt*QT, (qt+1)*QT), then proj + RS chunk qt."""
                nkb = 4 * qt + 4
                nwaves = 0
                attnT = atp.tile([P, 2, QT], bf16, tag="attnT", name=f"attnT_{qt}")
                for hp in range(2):
                    apair = psav.tile([HS + 1, QT], f32, tag="at", name=f"ap_{qt}_{hp}")
                    apodd = psav.tile([HS + 1, QT], f32, tag="at", name=f"ao_{qt}_{hp}")
                    for sub in range(QT // SUB):
                        qlo = qt * QT + sub * SUB
                        live = [kb for kb in range(nkb) if KB * kb < qlo + SUB]
                        n = len(live)
                        ssl = slice(sub * SUB, (sub + 1) * SUB)
                        waves = [live[i : i + 2] for i in range(0, n, 2)]
                        prev = None  # (ee, wave, i0)

                        def emit_av(ee, wave, i0):
                            for b, kb in enumerate(wave):
                                i = i0 + b
                                for par, aps in ((0, apair), (1, apodd)):
                                    he = (2 * hp + par) * (HS + 1)
                                    nc.tensor.matmul(
                                        aps[:, ssl],
                                        v4e[:, kb, he : he + HS + 1],
                                        ee[:, b, par, :],
                                        start=(i == 0),
                                        stop=(i == n - 1),
                                    )

                        i0 = 0
                        for w, wave in enumerate(waves):
                            nw = len(wave)
                            assert nw == 2, "live block counts are even"
                            sp = pssc.tile(
                                [P, 2, 2, SUB], f32, tag="sc",
                                name=f"sp_{qt}_{hp}_{sub}_{w}",
                            )
                            for b, kb in enumerate(wave):
                                ksl = slice(kb * KB, (kb + 1) * KB)
                                for par in range(2):
                                    nc.tensor.matmul(
                                        sp[:, b, par, :],
                                        k2T[:, par, hp, ksl],
                                        q2T[:, par, hp, qlo : qlo + SUB],
                                        start=True,
                                        stop=True,
                                    )
                                moff = KB * kb - qlo
                                if moff == 0:
                                    nc.vector.tensor_add(
                                        sp[:, b, :, 0:KB],
                                        sp[:, b, :, 0:KB],
                                        mtri_sb[:, None, :].to_broadcast((P, 2, KB)),
                                    )
                                elif moff == KB:
                                    nc.vector.tensor_add(
                                        sp[:, b, :, :],
                                        sp[:, b, :, :],
                                        mfull_sb[:, None, :].to_broadcast(
                                            (P, 2, SUB)
                                        ),
                                    )
                            if prev is not None:
                                emit_av(*prev)
                            ee = smallp.tile(
                                [P, 2, 2, SUB], bf16, tag="ee",
                                name=f"ee_{qt}_{hp}_{sub}_{w}",
                            )
                            nc.scalar.activation(
                                out=ee,
                                in_=sp,
                                func=mybir.ActivationFunctionType.Exp,
                                scale=float(HS) ** -0.5,
                            )
                            prev = (ee, wave, i0)
                            i0 += nw
                            nwaves += 1
                            # filler splicing starts several waves in so the
                            # filler's upstream deps (RS + LN1 vector chain)
                            # have landed and never stall the PE queue
                            if nwaves > fill_from and fillers:
                                fillers.pop(0)()
                        if prev is not None:
                            emit_av(*prev)
                    # normalize in SBUF off the PE critical path; both
                    # PSUM copies go first so the accumulator banks free
                    # up as fast as possible
                    uas = []
                    for par, aps in ((0, apair), (1, apodd)):
                        ua = normp.tile(
                            [HS + 1, QT], f32, tag="ua", name=f"ua_{qt}_{hp}_{par}"
                        )
                        nc.vector.tensor_copy(ua, aps)
                        uas.append(ua)
                    for par in range(2):
                        ua = uas[par]
                        rec = normp.tile([1, QT], f32, tag="rec", bufs=1)
                        nc.vector.reciprocal(rec, ua[HS : HS + 1, :])
                        bc = normp.tile([HS, QT], f32, tag="bc", bufs=1)
                        nc.gpsimd.partition_broadcast(bc, rec)
                        nc.vector.tensor_mul(
                            attnT[par * HS : (par + 1) * HS, hp, :],
                            ua[0:HS, :],
                            bc,
                        )
                if STAGE < 3:
                    return
                # proj for this qt's 4 token tiles, then RS chunk qt
                for mtl in range(4):
                    mt = 4 * qt + mtl
                    prj = projp.tile([P, D], bf16, tag="prj", name=f"prj_{mt}")
                    for nh in range(D // QT):
                        pps = psbig.tile(
                            [P, QT], f32, tag="big", name=f"pps_{mt}_{nh}"
                        )
                        for ko in range(2):
                            nc.tensor.matmul(
                                pps,
                                attnT[:, ko, mtl * P : (mtl + 1) * P],
                                wp_sb[:, ko, nh * QT : (nh + 1) * QT],
                                start=(ko == 0),
                                stop=(ko == 1),
                            )
                        nc.vector.tensor_copy(prj[:, nh * QT : (nh + 1) * QT], pps)
                    nc.sync.dma_start(
                        rs_in[qt].ap()[mtl * P : (mtl + 1) * P, :], prj
                    )
                nc.gpsimd.collective_compute(
                    "ReduceScatter",
                    mybir.AluOpType.add,
                    replica_groups=REPLICA_GROUPS,
                    ins=[rs_in[qt].ap().opt()],
                    outs=[rs_out[qt].ap().opt()],
                )

            # phase B pools (SBUF side; PSUM scratch opened after attention)
            # b2 broadcast lands in a transient f32 tile, kept only as bf16
            b2f = ln1p.tile([P, D], f32, tag="tmp", name="b2f")
            nc.gpsimd.dma_start(b2f, _bcast_row_ap(cvec.tensor, 5, D))
            nc.vector.tensor_copy(b2b, b2f)

            def ln1_vec(st):
                """rs_out[st] + xs[st] (b_proj pre-folded) -> LN1 -> x1r
                (bf16). Vector-only: the x1T transposes are emitted
                separately (ln1_tp) so they never head-of-line-block the
                PE queue behind a ReduceScatter."""
                yb = zp.tile([P, D], bf16, tag="z", name=f"yb_{st}")
                nc.gpsimd.dma_start(yb, rs_out[st].ap())
                y = ln1p.tile([P, D], f32, tag="y")
                nc.vector.tensor_copy(y, yb)
                xst = ln1p.tile([P, D], f32, tag="tmp", name=f"xst_{st}")
                nc.gpsimd.dma_start(xst, xs[st * P : (st + 1) * P, :])
                nc.vector.tensor_add(y, y, xst)
                stats = ln1p.tile([P, 2, 6], f32, tag="stats")
                yv = y.rearrange("p (s d) -> p s d", s=2)
                nc.vector.bn_stats(out=stats[:, 0, :], in_=yv[:, 0, :])
                nc.vector.bn_stats(out=stats[:, 1, :], in_=yv[:, 1, :])
                mv = ln1p.tile([P, 2], f32, tag="mv")
                nc.vector.bn_aggr(out=mv, in_=stats)
                rstd = ln1p.tile([P, 1], f32, tag="rstd")
                nc.scalar.activation(
                    out=rstd,
                    in_=mv[:, 1:2],
                    func=mybir.ActivationFunctionType.Sqrt,
                    bias=eps_t,
                    scale=1.0,
                )
                nc.vector.reciprocal(rstd, rstd)
                tmp = ln1p.tile([P, D], f32, tag="tmp")
                nc.vector.tensor_scalar(
                    out=tmp,
                    in0=y,
                    scalar1=mv[:, 0:1],
                    scalar2=rstd,
                    op0=mybir.AluOpType.subtract,
                    op1=mybir.AluOpType.mult,
                )
                nc.vector.tensor_mul(tmp, tmp, g1b)
                nc.vector.tensor_add(x1r[:, st, :], tmp, be1b)

            def ln1_tp(st, dks, tpool):
                """x1T transposes for token tile st, dk chunks `dks`."""
                for dk in dks:
                    tp = tpool.tile(
                        [P, P], bf16, tag="big", name=f"tp_{st}_{dk}",
                    )
                    nc.tensor.transpose(
                        tp, x1r[:, st, dk * P : (dk + 1) * P], identb
                    )
                    nc.vector.tensor_copy(x1T[:, dk, st * P : (st + 1) * P], tp)

            def ffn1_chunk(ft, st, nt, pool, relu_scalar=False):
                """FFN1 for the ft-th 128-wide hidden chunk over `nt` token
                tiles starting at tile st (N = nt*128); relu+bias on vector
                (filler mode) or scalar (phase B, where vector is loaded)."""
                in_sl = slice(st * P, (st + nt) * P)
                out_sl = slice((st % 2) * P, ((st % 2) + nt) * P)
                hps = pool.tile(
                    [P, nt * P], f32, tag="big",
                    padded_shape=[P, QT], name=f"hc_{ft}_{st}_{nt}",
                )
                for ko in range(D // P):
                    nc.tensor.matmul(
                        hps,
                        w1_sb[:, ko, ft * P : (ft + 1) * P],
                        x1T[:, ko, in_sl],
                        start=(ko == 0),
                        stop=(ko == D // P - 1),
                    )
                if relu_scalar:
                    nc.scalar.activation(
                        out=hT[:, ft, out_sl],
                        in_=hps,
                        func=mybir.ActivationFunctionType.Relu,
                        bias=b1_sb[:, ft : ft + 1],
                        scale=1.0,
                    )
                else:
                    nc.vector.tensor_scalar(
                        out=hT[:, ft, out_sl],
                        in0=hps,
                        scalar1=b1_sb[:, ft : ft + 1],
                        scalar2=0.0,
                        op0=mybir.AluOpType.add,
                        op1=mybir.AluOpType.max,
                    )

            def ln2_out(st, ps, eng=None):
                """z = ps + x1r[st] + b2 -> LN2 -> out for token tile st.
                `eng` carries the big elementwise ops (vector by default;
                gpsimd for one tile of each pair so the two LN2 chains of a
                pass run on different engines in parallel)."""
                if eng is None:
                    eng = nc.vector
                zm = zp.tile([P, D], bf16, tag="z", name=f"z_{st}")
                if eng is nc.gpsimd:
                    # gpsimd cannot read PSUM; scalar does the drain copy
                    nc.scalar.copy(zm, ps)
                else:
                    eng.tensor_copy(zm, ps)
                eng.tensor_add(zm, zm, x1r[:, st, :])
                eng.tensor_add(zm, zm, b2b)
                stats = ln1p.tile([P, 2, 6], f32, tag="stats")
                zv = zm.rearrange("p (s d) -> p s d", s=2)
                nc.vector.bn_stats(out=stats[:, 0, :], in_=zv[:, 0, :])
                nc.vector.bn_stats(out=stats[:, 1, :], in_=zv[:, 1, :])
                mv = ln1p.tile([P, 2], f32, tag="mv")
                nc.vector.bn_aggr(out=mv, in_=stats)
                rstd = ln1p.tile([P, 1], f32, tag="rstd")
                nc.scalar.activation(
                    out=rstd,
                    in_=mv[:, 1:2],
                    func=mybir.ActivationFunctionType.Sqrt,
                    bias=eps_t,
                    scale=1.0,
                )
                nc.vector.reciprocal(rstd, rstd)
                o = ln1p.tile([P, D], f32, tag="tmp", name=f"o_{st}")
                eng.tensor_scalar(
                    out=o,
                    in0=zm,
                    scalar1=mv[:, 0:1],
                    scalar2=rstd,
                    op0=mybir.AluOpType.subtract,
                    op1=mybir.AluOpType.mult,
                )
                eng.tensor_mul(o, o, g2b)
                eng.tensor_add(o, o, be2b)
                nc.sync.dma_start(out[st * P : (st + 1) * P, :], o)

            # ---- emission: QKV tiles interleaved with attention rows ----
            qkv_tile(0)
            deferred_consts()
            qkv_tile(1)
            if STAGE >= 2:
                attention(0)
            qkv_tile(2)
            if STAGE >= 2:
                attention(1)
                if STAGE >= 4:
                    # RS0 lands mid-attention(2). Even the LN1 vector chain
                    # is a late-spliced filler: emitted early it would
                    # head-of-line-block the vector queue (mask adds) on
                    # the RS0-gated rs_out read. Tile-0-only FFN1 chunks
                    # follow as PE density filler; LN1 of tile 1 splices in
                    # last (RS1 lands mid-attention(2)).
                    fillers.append(lambda: ln1_vec(0))
                    fillers.extend(
                        [lambda a=dk: ln1_tp(0, [a, a + 1], psbig)
                         for dk in range(0, D // P, 2)]
                    )
                    fillers.extend(
                        [lambda a=ft: ffn1_chunk(a, 0, 1, psbig)
                         for ft in range(8)]
                    )
                    fillers.append(lambda: ln1_vec(1))
                    fillers.extend(
                        [lambda a=dk: ln1_tp(1, [a, a + 1], psbig)
                         for dk in range(0, D // P, 2)]
                    )
            qkv_tile(3)
            if STAGE >= 2:
                attention(2, fill_from=4)
                if STAGE >= 4:
                    fillers.extend(
                        [lambda a=ft: ffn1_chunk(a, 1, 1, psbig)
                         for ft in range(8)]
                    )
                    fillers.extend(
                        [lambda a=ft: ffn1_chunk(a, 0, 2, psbig)
                         for ft in range(8, F // P)]
                    )

                # QKV SBUF pools close here, in attention(3)'s shadow; the
                # w2 streaming pool reuses their space. PSUM pools stay
                # open through phase B (psbig scratch, pssc FFN2 accum) so
                # no PSUM drain bubbles are inserted.
                xr_cm.__exit__(None, None, None)
                wqkv_cm.__exit__(None, None, None)
                w2s_cm = tc.tile_pool(name="w2s", bufs=4)
                w2s = w2s_cm.__enter__()

                attention(3, fill_from=4)
                # flush any filler items the attention waves didn't absorb
                while fillers:
                    fillers.pop(0)()
            else:
                xr_cm.__exit__(None, None, None)
                wqkv_cm.__exit__(None, None, None)
                w2s_cm = tc.tile_pool(name="w2s", bufs=3)
                w2s = w2s_cm.__enter__()

            if STAGE >= 4:
                # ---- phase B: sequential behind the RS chunks ----
                def ffn2_pass(h):
                    """FFN2 for token tiles 2h, 2h+1: stream w2 in 256-row
                    chunks alternating two DMA rings, hT stationary, two
                    N=512 matmuls per k-chunk per tile, then LN2 + out."""
                    psA = pssc.tile([P, D], f32, tag="sc", name=f"psA_{h}")
                    psB = pssc.tile([P, D], f32, tag="sc", name=f"psB_{h}")
                    nk = F // (2 * P)
                    for c in range(nk):
                        w2c = w2s.tile(
                            [P, 2, D], bf16, tag="w2c", name=f"w2c_{h}_{c}"
                        )
                        ring = nc.sync if c % 2 == 0 else nc.scalar
                        ring.dma_start(
                            w2c,
                            w2[c * 2 * P : (c + 1) * 2 * P, :].rearrange(
                                "(kk p) n -> p kk n", p=P
                            ),
                        )
                        for kk in range(2):
                            k = 2 * c + kk
                            for t, ps in ((0, psA), (1, psB)):
                                csl = slice(t * P, (t + 1) * P)
                                for dh in range(2):
                                    nc.tensor.matmul(
                                        ps[:, dh * QT : (dh + 1) * QT],
                                        hT[:, k, csl],
                                        w2c[:, kk, dh * QT : (dh + 1) * QT],
                                        start=(k == 0),
                                        stop=(k == 2 * nk - 1),
                                    )
                    ln2_out(2 * h + 0, psA)
                    ln2_out(2 * h + 1, psB, nc.gpsimd)

                ln1_vec(2)
                ln1_tp(2, range(D // P), psbig)
                # pass(0) must consume tiles {0,1} of hT before the tile-2
                # chunks overwrite their columns (hT holds 2 tiles at once)
                ffn2_pass(0)
                for ft in range(F // P):
                    ffn1_chunk(ft, 2, 1, psbig, relu_scalar=True)
                ln1_vec(3)
                ln1_tp(3, range(D // P), psbig)
                for ft in range(F // P):
                    ffn1_chunk(ft, 3, 1, psbig, relu_scalar=True)
                ffn2_pass(1)
            else:
                dummy = ln1p.tile([P, D], f32, tag="tmp", name="dummy")
                nc.vector.memset(dummy, 0.0)
                for st in range(SL // P):
                    nc.sync.dma_start(out[st * P : (st + 1) * P, :], dummy)

            av_cm.__exit__(None, None, None)
            sc_cm.__exit__(None, None, None)
            big_cm.__exit__(None, None, None)
            w2s_cm.__exit__(None, None, None)
            ln1p_cm.__exit__(None, None, None)
            zp_cm.__exit__(None, None, None)
            projp_cm.__exit__(None, None, None)
            normp_cm.__exit__(None, None, None)
            smallp_cm.__exit__(None, None, None)
            atp_cm.__exit__(None, None, None)
            keep_cm.__exit__(None, None, None)

    nc.compile()
    return nc


_NC_CACHE = []


def _get_nc():
    if not _NC_CACHE:
        _NC_CACHE.append(build_bass())
    return _NC_CACHE[0]


def _token_blocks(r):
    """Global token rows (within a batch element) owned by rank r, as NCK
    blocks of 128: block ck covers rows [512*ck + 128*r, 512*ck + 128*r + 128)."""
    return [slice(QT * ck + P * r, QT * ck + P * r + P) for ck in range(NCK)]


def make_in_maps(x, wq, wk, wv, w_proj, b_proj, w1, b1, w2, b2, g1, be1, g2, be2):
    x = np.asarray(x, dtype=np.float32)
    bp = np.asarray(b_proj, dtype=np.float32)
    cat = lambda w, h0: np.ascontiguousarray(
        np.concatenate(
            [np.asarray(w[h0 + i], dtype=np.float32) for i in range(HPC)], axis=1
        )
    ).astype(bfnp)
    cvec_rows = [b_proj, g1, be1, g2, be2, b2]
    cvec = np.ascontiguousarray(
        np.stack([np.asarray(v, dtype=np.float32) for v in cvec_rows])
    )
    w1c = np.ascontiguousarray(np.asarray(w1, dtype=np.float32)).astype(bfnp)
    w2c = np.ascontiguousarray(np.asarray(w2, dtype=np.float32)).astype(bfnp)
    b1c = np.ascontiguousarray(np.asarray(b1, dtype=np.float32))
    wpc = np.asarray(w_proj, dtype=np.float32)
    xTs = [np.ascontiguousarray(x[g].T.astype(bfnp)) for g in range(B)]
    in_maps = []
    for c in range(NCORES):
        g, r = divmod(c, TP)
        h0 = HPC * r
        # b_proj is folded into the residual here instead of on device
        xs_blocks = np.concatenate(
            [x[g, blk] + bp for blk in _token_blocks(r)], axis=0
        )
        in_maps.append(
            {
                "xT": xTs[g],
                "xs": np.ascontiguousarray(xs_blocks),
                "wq2": cat(wq, h0),
                "wk2": cat(wk, h0),
                "wv4": cat(wv, h0),
                "wp": np.ascontiguousarray(
                    wpc[HPC * HS * r : HPC * HS * (r + 1)].astype(bfnp)
                ),
                "w1": w1c,
                "w2": w2c,
                "cvec": cvec,
                "b1d": b1c,
            }
        )
    return in_maps


def assemble(results):
    full = np.empty((B, S, D), dtype=np.float32)
    for c in range(NCORES):
        g, r = divmod(c, TP)
        o = results[c]["out"]
        for ck, blk in enumerate(_token_blocks(r)):
            full[g, blk] = o[ck * P : (ck + 1) * P]
    return full


def kernel(**inputs):
    nc = _get_nc()
    in_maps = make_in_maps(**inputs)
    res = run_bass_kernel_spmd(nc, in_maps, core_ids=list(range(NCORES)))
    return assemble(res.results)


# revision 58
# speedup vs baseline: 1.0784x; 1.0784x over previous
"""Trainium2 Bass kernel for nn_Block_77318001263203 (dense transformer block).

Distribution over 8 NeuronCores: data-parallel over batch (2 groups of 4
cores) x tensor-parallel over heads (4 heads/core) for attention+proj,
4-way-chunked bf16 ReduceScatter of the proj partials over each 4-core
group (each chunk hands every rank one 128-token block, so rank r owns
the strided token set {512*ck + 128*r + j}), then token-parallel FFN with
replicated FFN weights — no second collective.

v3 schedule (vs v2): built for PE clock density — the HAM activity
monitor halves the PE clock whenever its duty cycle sags, and v2 spent
~40% of its span at 1.2 GHz.
  - PE warm-up: a burst of identity transposes runs during the initial
    weight DMAs so the first real matmuls start at 2.4 GHz.
  - Startup DMAs spread over four rings (sync/scalar/vector/gpsimd).
  - QKV token tiles and attention row blocks interleave (tt0, tt1, qt0,
    tt2, qt1, tt3, qt2, qt3) so attention's scalar-engine exp stream
    starts ~40us earlier and overlaps QKV matmuls.
  - Attention processes key blocks in 2-block waves with one fused exp
    per wave (N=1024 activation ops instead of N=512) and the AV matmuls
    trailing one wave for latency hiding.
  - FFN1 runs token tiles {0,1} as one N=256 pass (vector-relu), tiles
    2,3 as N=128 passes (scalar-relu); FFN2 streams w2 row-chunks once
    per token-tile pair with hT stationary and N=512 moving, accumulating
    both tiles' [128,1024] outputs per chunk.
  - Phase B is sequential after the last proj chunk: the ReduceScatter
    latency hides behind the {0,1}-pair FFN work and the tail after RS3
    lands is just tile 3's ln1 -> ffn1 -> ffn2 chain.

All matmuls bf16 (fp32 PSUM accumulate); b_proj is folded into the xs
residual host-side. Known HW faults respected from v2: no bf16 matmul
operands at partition offset 64, no f32r tile_position packing.

kernel(**inputs) takes the FULL inputs from setup_inputs() and returns the
FULL [2, 2048, 1024] float32 output.
"""

import numpy as np
import ml_dtypes

import concourse.bass as bass
import concourse.mybir as mybir
import concourse.tile as tile
from concourse import bacc
from concourse.bass_utils import run_bass_kernel_spmd
from concourse.masks import make_identity

# problem dims (hardcoded per the harness contract)
B, S, D = 2, 2048, 1024
H, HS, F = 16, 64, 4096
EPS = 1e-5
P = 128
NCORES = 8
TP = 4  # cores per batch group
HPC = H // TP  # heads per core = 4
SL = S // TP  # tokens owned per core = 512 (4 strided blocks of 128)
QT = 512  # query row tile (attention row granularity)
SUB = 256  # score/exp subtile width
KB = 128  # key block
NCK = 4  # reduce-scatter chunks
NEG = -1.0e9  # additive causal mask (exp underflows to exactly 0)

f32 = mybir.dt.float32
bf16 = mybir.dt.bfloat16
bfnp = ml_dtypes.bfloat16

REPLICA_GROUPS = [[0, 1, 2, 3], [4, 5, 6, 7]]


def _bcast_row_ap(t, row, width):
    """DMA-source AP broadcasting row `row` of DRAM tensor t to 128 partitions."""
    return bass.AP(tensor=t, offset=row * width, ap=[[0, P], [1, width]])


def build_bass():
    import os

    # debug bisection: 1=QKV, 2=+attention rows, 3=+proj/RS, 4=full
    STAGE = int(os.environ.get("KSTAGE", "4"))
    nc = bacc.Bacc("TRN2", target_bir_lowering=False, debug=False, num_devices=NCORES)

    xT = nc.dram_tensor("xT", [D, S], bf16, kind="ExternalInput").ap()
    xs = nc.dram_tensor("xs", [SL, D], f32, kind="ExternalInput").ap()
    wq2 = nc.dram_tensor("wq2", [D, HPC * HS], bf16, kind="ExternalInput").ap()
    wk2 = nc.dram_tensor("wk2", [D, HPC * HS], bf16, kind="ExternalInput").ap()
    wv4 = nc.dram_tensor("wv4", [D, HPC * HS], bf16, kind="ExternalInput").ap()
    wp = nc.dram_tensor("wp", [HPC * HS, D], bf16, kind="ExternalInput").ap()
    w1 = nc.dram_tensor("w1", [D, F], bf16, kind="ExternalInput").ap()
    w2 = nc.dram_tensor("w2", [F, D], bf16, kind="ExternalInput").ap()
    cvec = nc.dram_tensor("cvec", [6, D], f32, kind="ExternalInput").ap()
    b1d = nc.dram_tensor("b1d", [F], f32, kind="ExternalInput").ap()
    out = nc.dram_tensor("out", [SL, D], f32, kind="ExternalOutput").ap()

    # per-chunk collective bounce buffers (separate tensors -> precise deps)
    rs_in = [nc.dram_tensor(f"rs_in{c}", [QT, D], bf16) for c in range(NCK)]
    rs_out = [nc.dram_tensor(f"rs_out{c}", [P, D], bf16) for c in range(NCK)]
    # tiny dummy collective fired at kernel start: absorbs the CC stream's
    # first-trigger startup latency (~11us) so RS chunk 0 starts promptly
    cc_warm_in = nc.dram_tensor("cc_warm_in", [TP, 1], bf16)
    cc_warm_out = nc.dram_tensor("cc_warm_out", [1, 1], bf16)

    # additive causal mask [all-NEG block | lower-triangle-NEG block]:
    # mfull[:, KB:] is the triangle alone (keep 0 where key t <= query q).
    tri = np.where(
        np.arange(KB)[:, None] <= np.arange(KB)[None, :], 0.0, NEG
    ).astype(np.float32)
    full = np.concatenate([np.full((KB, KB), NEG, np.float32), tri], axis=1)
    m_full_dram = nc.inline_tensor(np.ascontiguousarray(full), name="mask_full")

    with tile.TileContext(nc) as tc:
        with tc.tile_pool(name="const", bufs=1) as constp:
            identb = constp.tile([P, P], bf16)
            make_identity(nc, identb)
            eps_t = constp.tile([P, 1], f32)
            nc.vector.memset(eps_t, EPS)
            # const tiles allocated here; their broadcast DMAs (which write
            # 128x the source bytes into SBUF) are deferred until after the
            # startup-critical QKV loads are in the rings
            b1_sb = constp.tile([P, F // P], f32)
            g1b = constp.tile([P, D], f32)
            be1b = constp.tile([P, D], f32)
            g2b = constp.tile([P, D], f32)
            be2b = constp.tile([P, D], f32)
            b2b = constp.tile([P, D], bf16)

            keep_cm = tc.tile_pool(name="keep", bufs=1)
            keep = keep_cm.__enter__()
            mfull_sb = keep.tile([P, 2 * KB], f32, tag="mfull")
            mtri_sb = mfull_sb[:, KB : 2 * KB]

            # wp's dma is emitted after wq below (same ring, wq is urgent)
            wp_sb = keep.tile([P, (HPC * HS) // P, D], bf16, tag="wp")

            # attention working set. q/k live on partitions 0-63 with the
            # head parity in the free dim: bf16 matmul operands fault on
            # HW at partition offset 64, so every score matmul reads
            # partition-offset-0 slices.
            q2T = keep.tile([HS, 2, 2, S], bf16, tag="q2T")
            k2T = keep.tile([HS, 2, 2, S], bf16, tag="k2T")
            v4e = keep.tile([P, S // P, HPC * (HS + 1)], bf16, tag="v4e")
            ones4 = keep.tile([P, HPC, 1], bf16, tag="ones4")
            nc.vector.memset(ones4, 1.0)

            # phase B persistents
            w1_sb = keep.tile([P, D // P, F], bf16, tag="w1")
            x1T = keep.tile([P, D // P, SL], bf16, tag="x1T")
            x1r = keep.tile([P, SL // P, D], bf16, tag="x1r")
            hT = keep.tile([P, F // P, SL // 2], bf16, tag="hT")

            # PE warm-up: ~48 back-to-back transposes (~5us of PE activity)
            # run during the initial weight DMAs so the HAM clock gate is
            # released before the first real matmul.
            with tc.tile_pool(name="ps_warm", bufs=1, space="PSUM") as pswarm:
                wtile = pswarm.tile([P, P], bf16, tag="warm")
                for _ in range(48):
                    nc.tensor.transpose(wtile, identb, identb)
            if STAGE >= 3:
                nc.gpsimd.collective_compute(
                    "ReduceScatter",
                    mybir.AluOpType.add,
                    replica_groups=REPLICA_GROUPS,
                    ins=[cc_warm_in.ap().opt()],
                    outs=[cc_warm_out.ap().opt()],
                )

            # ---------------- interleaved QKV + attention ----------------
            # SBUF pool stack: pools that outlive QKV open first so the
            # QKV pools (wqkv, xr) can close mid-kernel in LIFO order.
            atp_cm = tc.tile_pool(name="atp", bufs=1)
            atp = atp_cm.__enter__()
            smallp_cm = tc.tile_pool(name="smallp", bufs=2)
            smallp = smallp_cm.__enter__()
            normp_cm = tc.tile_pool(name="normp", bufs=2)
            normp = normp_cm.__enter__()
            projp_cm = tc.tile_pool(name="projp", bufs=1)
            projp = projp_cm.__enter__()
            zp_cm = tc.tile_pool(name="zp", bufs=2)
            zp = zp_cm.__enter__()
            ln1p_cm = tc.tile_pool(name="ln1p", bufs=1)
            ln1p = ln1p_cm.__enter__()

            # QKV weights in ko-half loads so the first accumulation chunks
            # can begin before the full weight tile lands
            wqkv_cm = tc.tile_pool(name="wqkv", bufs=1)
            wqkvp = wqkv_cm.__enter__()
            wq_sb = wqkvp.tile([P, D // P, HPC * HS], bf16, tag="wq")
            wk_sb = wqkvp.tile([P, D // P, HPC * HS], bf16, tag="wk")
            wv_sb = wqkvp.tile([P, D // P, HPC * HS], bf16, tag="wv")
            for h in range(4):
                hsl = slice(h * 2, (h + 1) * 2)
                rsl = slice(h * (D // 4), (h + 1) * (D // 4))
                nc.scalar.dma_start(
                    wq_sb[:, hsl, :],
                    wq2[rsl, :].rearrange("(ko p) m -> p ko m", p=P),
                )
                nc.gpsimd.dma_start(
                    wk_sb[:, hsl, :],
                    wk2[rsl, :].rearrange("(ko p) m -> p ko m", p=P),
                )
                nc.gpsimd.dma_start(
                    wv_sb[:, hsl, :],
                    wv4[rsl, :].rearrange("(ko p) m -> p ko m", p=P),
                )
            nc.scalar.dma_start(wp_sb, wp.rearrange("(ko p) n -> p ko n", p=P))

            def deferred_consts():
                """Const loads emitted behind the urgent QKV ring traffic."""
                nc.sync.dma_start(mfull_sb, m_full_dram.ap())
                nc.sync.dma_start(b1_sb, b1d.rearrange("(ko p) -> p ko", p=P))
                nc.sync.dma_start(g1b, _bcast_row_ap(cvec.tensor, 1, D))
                nc.sync.dma_start(be1b, _bcast_row_ap(cvec.tensor, 2, D))
                nc.sync.dma_start(g2b, _bcast_row_ap(cvec.tensor, 3, D))
                nc.sync.dma_start(be2b, _bcast_row_ap(cvec.tensor, 4, D))

            xr_cm = tc.tile_pool(name="xrp", bufs=1)
            xrp = xr_cm.__enter__()
            big_cm = tc.tile_pool(name="ps_big", bufs=2, space="PSUM")
            psbig = big_cm.__enter__()
            sc_cm = tc.tile_pool(name="ps_sc", bufs=2, space="PSUM")
            pssc = sc_cm.__enter__()
            av_cm = tc.tile_pool(name="ps_av", bufs=2, space="PSUM")
            psav = av_cm.__enter__()

            def qkv_tile(tt):
                xr = xrp.tile([P, D // P, QT], bf16, tag="xr", name=f"xr_{tt}")
                # quarter-loads so the first ko chunks' matmuls can start
                # before the whole tile has landed
                for h in range(4):
                    nc.sync.dma_start(
                        xr[:, h * 2 : (h + 1) * 2, :],
                        xT[h * (D // 4) : (h + 1) * (D // 4),
                           tt * QT : (tt + 1) * QT].rearrange(
                            "(ko p) m -> p ko m", p=P
                        ),
                    )
                for hp in range(2):
                    qps = psbig.tile([P, QT], f32, tag="big", name=f"qps_{tt}_{hp}")
                    for ko in range(D // P):
                        nc.tensor.matmul(
                            qps,
                            wq_sb[:, ko, hp * P : (hp + 1) * P],
                            xr[:, ko, :],
                            start=(ko == 0),
                            stop=(ko == D // P - 1),
                        )
                    for par in range(2):
                        nc.vector.tensor_copy(
                            q2T[:, par, hp, tt * QT : (tt + 1) * QT],
                            qps[par * HS : (par + 1) * HS, :],
                        )
                    kps = psbig.tile([P, QT], f32, tag="big", name=f"kps_{tt}_{hp}")
                    for ko in range(D // P):
                        nc.tensor.matmul(
                            kps,
                            wk_sb[:, ko, hp * P : (hp + 1) * P],
                            xr[:, ko, :],
                            start=(ko == 0),
                            stop=(ko == D // P - 1),
                        )
                    for par in range(2):
                        nc.vector.tensor_copy(
                            k2T[:, par, hp, tt * QT : (tt + 1) * QT],
                            kps[par * HS : (par + 1) * HS, :],
                        )
                for mt in range(QT // P):
                    vps = psbig.tile(
                        [P, HPC * HS], f32, tag="big",
                        padded_shape=[P, QT], name=f"vps_{tt}_{mt}",
                    )
                    for ko in range(D // P):
                        nc.tensor.matmul(
                            vps,
                            xr[:, ko, mt * P : (mt + 1) * P],
                            wv_sb[:, ko, :],
                            start=(ko == 0),
                            stop=(ko == D // P - 1),
                        )
                    idx = tt * (QT // P) + mt
                    vv = v4e[:, idx, :].rearrange("p (h e) -> p h e", e=HS + 1)
                    nc.vector.tensor_copy(
                        vv[:, :, 0:HS],
                        vps.rearrange("p (h e) -> p h e", e=HS),
                    )
                    nc.vector.tensor_copy(vv[:, :, HS : HS + 1], ones4)
                # the big w1 load rides behind the latency-critical QKV
                # loads in 2MB chunks (needed only from attention(3) on)
                nc.scalar.dma_start(
                    w1_sb[:, :, tt * (F // 4) : (tt + 1) * (F // 4)],
                    w1[:, tt * (F // 4) : (tt + 1) * (F // 4)].rearrange(
                        "(ko p) m -> p ko m", p=P
                    ),
                )

            # PE filler work items (thunks) spliced between attention waves:
            # keeps the PE duty cycle high through the scalar-bound exp
            # stream so the HAM clock gate stays released, and moves phase-B
            # matmuls off the critical tail.
            fillers = []

            def attention(qt, fill_from=10 ** 9):
                """Score/exp/AV waves + normalize for query rows
                [qt*QT, (qt+1)*QT), then proj + RS chunk qt."""
                nkb = 4 * qt + 4
                nwaves = 0
                attnT = atp.tile([P, 2, QT], bf16, tag="attnT", name=f"attnT_{qt}")
                for hp in range(2):
                    apair = psav.tile([HS + 1, QT], f32, tag="at", name=f"ap_{qt}_{hp}")
                    apodd = psav.tile([HS + 1, QT], f32, tag="at", name=f"ao_{qt}_{hp}")
                    for sub in range(QT // SUB):
                        qlo = qt * QT + sub * SUB
                        live = [kb for kb in range(nkb) if KB * kb < qlo + SUB]
                        n = len(live)
                        ssl = slice(sub * SUB, (sub + 1) * SUB)
                        waves = [live[i : i + 2] for i in range(0, n, 2)]
                        prev = None  # (ee, wave, i0)

                        def emit_av(ee, wave, i0):
                            for b, kb in enumerate(wave):
                                i = i0 + b
                                for par, aps in ((0, apair), (1, apodd)):
                                    he = (2 * hp + par) * (HS + 1)
                                    nc.tensor.matmul(
                                        aps[:, ssl],
                                        v4e[:, kb, he : he + HS + 1],
                                        ee[:, b, par, :],
                                        start=(i == 0),
                                        stop=(i == n - 1),
                                    )

                        i0 = 0
                        for w, wave in enumerate(waves):
                            nw = len(wave)
                            assert nw == 2, "live block counts are even"
                            sp = pssc.tile(
                                [P, 2, 2, SUB], f32, tag="sc",
                                name=f"sp_{qt}_{hp}_{sub}_{w}",
                            )
                            for b, kb in enumerate(wave):
                                ksl = slice(kb * KB, (kb + 1) * KB)
                                for par in range(2):
                                    nc.tensor.matmul(
                                        sp[:, b, par, :],
                                        k2T[:, par, hp, ksl],
                                        q2T[:, par, hp, qlo : qlo + SUB],
                                        start=True,
                                        stop=True,
                                    )
                                moff = KB * kb - qlo
                                if moff == 0:
                                    nc.vector.tensor_add(
                                        sp[:, b, :, 0:KB],
                                        sp[:, b, :, 0:KB],
                                        mtri_sb[:, None, :].to_broadcast((P, 2, KB)),
                                    )
                                elif moff == KB:
                                    nc.vector.tensor_add(
                                        sp[:, b, :, :],
                                        sp[:, b, :, :],
                                        mfull_sb[:, None, :].to_broadcast(
                                            (P, 2, SUB)
                                        ),
                                    )
                            if prev is not None:
                                emit_av(*prev)
                            ee = smallp.tile(
                                [P, 2, 2, SUB], bf16, tag="ee",
                                name=f"ee_{qt}_{hp}_{sub}_{w}",
                            )
                            nc.scalar.activation(
                                out=ee,
                                in_=sp,
                                func=mybir.ActivationFunctionType.Exp,
                                scale=float(HS) ** -0.5,
                            )
                            prev = (ee, wave, i0)
                            i0 += nw
                            nwaves += 1
                            # filler splicing starts several waves in so the
                            # filler's upstream deps (RS + LN1 vector chain)
                            # have landed and never stall the PE queue
                            if nwaves > fill_from and fillers:
                                fillers.pop(0)()
                        if prev is not None:
                            emit_av(*prev)
                    # normalize in SBUF off the PE critical path; both
                    # PSUM copies go first so the accumulator banks free
                    # up as fast as possible
                    uas = []
                    for par, aps in ((0, apair), (1, apodd)):
                        ua = normp.tile(
                            [HS + 1, QT], f32, tag="ua", name=f"ua_{qt}_{hp}_{par}"
                        )
                        nc.vector.tensor_copy(ua, aps)
                        uas.append(ua)
                    for par in range(2):
                        ua = uas[par]
                        rec = normp.tile([1, QT], f32, tag="rec", bufs=1)
                        nc.vector.reciprocal(rec, ua[HS : HS + 1, :])
                        bc = normp.tile([HS, QT], f32, tag="bc", bufs=1)
                        nc.gpsimd.partition_broadcast(bc, rec)
                        nc.vector.tensor_mul(
                            attnT[par * HS : (par + 1) * HS, hp, :],
                            ua[0:HS, :],
                            bc,
                        )
                if STAGE < 3:
                    return
                # proj for this qt's 4 token tiles, then RS chunk qt
                for mtl in range(4):
                    mt = 4 * qt + mtl
                    prj = projp.tile([P, D], bf16, tag="prj", name=f"prj_{mt}")
                    for nh in range(D // QT):
                        pps = psbig.tile(
                            [P, QT], f32, tag="big", name=f"pps_{mt}_{nh}"
                        )
                        for ko in range(2):
                            nc.tensor.matmul(
                                pps,
                                attnT[:, ko, mtl * P : (mtl + 1) * P],
                                wp_sb[:, ko, nh * QT : (nh + 1) * QT],
                                start=(ko == 0),
                                stop=(ko == 1),
                            )
                        nc.vector.tensor_copy(prj[:, nh * QT : (nh + 1) * QT], pps)
                    nc.sync.dma_start(
                        rs_in[qt].ap()[mtl * P : (mtl + 1) * P, :], prj
                    )
                nc.gpsimd.collective_compute(
                    "ReduceScatter",
                    mybir.AluOpType.add,
                    replica_groups=REPLICA_GROUPS,
                    ins=[rs_in[qt].ap().opt()],
                    outs=[rs_out[qt].ap().opt()],
                )

            # phase B pools (SBUF side; PSUM scratch opened after attention)
            # b2 broadcast lands in a transient f32 tile, kept only as bf16
            b2f = ln1p.tile([P, D], f32, tag="tmp", name="b2f")
            nc.gpsimd.dma_start(b2f, _bcast_row_ap(cvec.tensor, 5, D))
            nc.vector.tensor_copy(b2b, b2f)

            def ln1_vec(st):
                """rs_out[st] + xs[st] (b_proj pre-folded) -> LN1 -> x1r
                (bf16). Vector-only: the x1T transposes are emitted
                separately (ln1_tp) so they never head-of-line-block the
                PE queue behind a ReduceScatter."""
                yb = zp.tile([P, D], bf16, tag="z", name=f"yb_{st}")
                nc.gpsimd.dma_start(yb, rs_out[st].ap())
                y = ln1p.tile([P, D], f32, tag="y")
                nc.vector.tensor_copy(y, yb)
                xst = ln1p.tile([P, D], f32, tag="tmp", name=f"xst_{st}")
                nc.gpsimd.dma_start(xst, xs[st * P : (st + 1) * P, :])
                nc.vector.tensor_add(y, y, xst)
                stats = ln1p.tile([P, 2, 6], f32, tag="stats")
                yv = y.rearrange("p (s d) -> p s d", s=2)
                nc.vector.bn_stats(out=stats[:, 0, :], in_=yv[:, 0, :])
                nc.vector.bn_stats(out=stats[:, 1, :], in_=yv[:, 1, :])
                mv = ln1p.tile([P, 2], f32, tag="mv")
                nc.vector.bn_aggr(out=mv, in_=stats)
                rstd = ln1p.tile([P, 1], f32, tag="rstd")
                nc.scalar.activation(
                    out=rstd,
                    in_=mv[:, 1:2],
                    func=mybir.ActivationFunctionType.Sqrt,
                    bias=eps_t,
                    scale=1.0,
                )
                nc.vector.reciprocal(rstd, rstd)
                tmp = ln1p.tile([P, D], f32, tag="tmp")
                nc.vector.tensor_scalar(
                    out=tmp,
                    in0=y,
                    scalar1=mv[:, 0:1],
                    scalar2=rstd,
                    op0=mybir.AluOpType.subtract,
                    op1=mybir.AluOpType.mult,
                )
                nc.vector.tensor_mul(tmp, tmp, g1b)
                nc.vector.tensor_add(x1r[:, st, :], tmp, be1b)

            def ln1_tp(st, dks, tpool):
                """x1T transposes for token tile st, dk chunks `dks`."""
                for dk in dks:
                    tp = tpool.tile(
                        [P, P], bf16, tag="big", name=f"tp_{st}_{dk}",
                    )
                    nc.tensor.transpose(
                        tp, x1r[:, st, dk * P : (dk + 1) * P], identb
                    )
                    nc.vector.tensor_copy(x1T[:, dk, st * P : (st + 1) * P], tp)

            def ffn1_chunk(ft, st, nt, pool, relu_scalar=False):
                """FFN1 for the ft-th 128-wide hidden chunk over `nt` token
                tiles starting at tile st (N = nt*128); relu+bias on vector
                (filler mode) or scalar (phase B, where vector is loaded)."""
                in_sl = slice(st * P, (st + nt) * P)
                out_sl = slice((st % 2) * P, ((st % 2) + nt) * P)
                hps = pool.tile(
                    [P, nt * P], f32, tag="big",
                    padded_shape=[P, QT], name=f"hc_{ft}_{st}_{nt}",
                )
                for ko in range(D // P):
                    nc.tensor.matmul(
                        hps,
                        w1_sb[:, ko, ft * P : (ft + 1) * P],
                        x1T[:, ko, in_sl],
                        start=(ko == 0),
                        stop=(ko == D // P - 1),
                    )
                if relu_scalar:
                    nc.scalar.activation(
                        out=hT[:, ft, out_sl],
                        in_=hps,
                        func=mybir.ActivationFunctionType.Relu,
                        bias=b1_sb[:, ft : ft + 1],
                        scale=1.0,
                    )
                else:
                    nc.vector.tensor_scalar(
                        out=hT[:, ft, out_sl],
                        in0=hps,
                        scalar1=b1_sb[:, ft : ft + 1],
                        scalar2=0.0,
                        op0=mybir.AluOpType.add,
                        op1=mybir.AluOpType.max,
                    )

            def ln2_out(st, ps, eng=None):
                """z = ps + x1r[st] + b2 -> LN2 -> out for token tile st.
                Odd tiles drain PSUM via the scalar engine so the two
                drains of a pass run in parallel."""
                eng = nc.vector
                zm = zp.tile([P, D], bf16, tag="z", name=f"z_{st}")
                if st % 2 == 1:
                    nc.scalar.copy(zm, ps)
                else:
                    eng.tensor_copy(zm, ps)
                eng.tensor_add(zm, zm, x1r[:, st, :])
                eng.tensor_add(zm, zm, b2b)
                stats = ln1p.tile([P, 2, 6], f32, tag="stats")
                zv = zm.rearrange("p (s d) -> p s d", s=2)
                nc.vector.bn_stats(out=stats[:, 0, :], in_=zv[:, 0, :])
                nc.vector.bn_stats(out=stats[:, 1, :], in_=zv[:, 1, :])
                mv = ln1p.tile([P, 2], f32, tag="mv")
                nc.vector.bn_aggr(out=mv, in_=stats)
                rstd = ln1p.tile([P, 1], f32, tag="rstd")
                nc.scalar.activation(
                    out=rstd,
                    in_=mv[:, 1:2],
                    func=mybir.ActivationFunctionType.Sqrt,
                    bias=eps_t,
                    scale=1.0,
                )
                nc.vector.reciprocal(rstd, rstd)
                o = ln1p.tile([P, D], f32, tag="tmp", name=f"o_{st}")
                eng.tensor_scalar(
                    out=o,
                    in0=zm,
                    scalar1=mv[:, 0:1],
                    scalar2=rstd,
                    op0=mybir.AluOpType.subtract,
                    op1=mybir.AluOpType.mult,
                )
                eng.tensor_mul(o, o, g2b)
                eng.tensor_add(o, o, be2b)
                nc.sync.dma_start(out[st * P : (st + 1) * P, :], o)

            # ---- emission: QKV tiles interleaved with attention rows ----
            qkv_tile(0)
            deferred_consts()
            qkv_tile(1)
            if STAGE >= 2:
                attention(0)
            qkv_tile(2)
            if STAGE >= 2:
                attention(1)
                if STAGE >= 4:
                    # RS0 lands mid-attention(2). Even the LN1 vector chain
                    # is a late-spliced filler: emitted early it would
                    # head-of-line-block the vector queue (mask adds) on
                    # the RS0-gated rs_out read. Tile-0-only FFN1 chunks
                    # follow as PE density filler; LN1 of tile 1 splices in
                    # last (RS1 lands mid-attention(2)).
                    fillers.append(lambda: ln1_vec(0))
                    fillers.extend(
                        [lambda a=dk: ln1_tp(0, [a, a + 1], psbig)
                         for dk in range(0, D // P, 2)]
                    )
                    fillers.extend(
                        [lambda a=ft: ffn1_chunk(a, 0, 1, psbig)
                         for ft in range(8)]
                    )
                    fillers.append(lambda: ln1_vec(1))
                    fillers.extend(
                        [lambda a=dk: ln1_tp(1, [a, a + 1], psbig)
                         for dk in range(0, D // P, 2)]
                    )
            qkv_tile(3)
            if STAGE >= 2:
                attention(2, fill_from=4)
                if STAGE >= 4:
                    fillers.extend(
                        [lambda a=ft: ffn1_chunk(a, 1, 1, psbig)
                         for ft in range(8)]
                    )
                    fillers.extend(
                        [lambda a=ft: ffn1_chunk(a, 0, 2, psbig)
                         for ft in range(8, F // P)]
                    )

                # QKV SBUF pools close here, in attention(3)'s shadow; the
                # w2 streaming pool reuses their space. PSUM pools stay
                # open through phase B (psbig scratch, pssc FFN2 accum) so
                # no PSUM drain bubbles are inserted.
                xr_cm.__exit__(None, None, None)
                wqkv_cm.__exit__(None, None, None)
                w2s_cm = tc.tile_pool(name="w2s", bufs=4)
                w2s = w2s_cm.__enter__()

                attention(3, fill_from=4)
                # flush any filler items the attention waves didn't absorb
                while fillers:
                    fillers.pop(0)()
            else:
                xr_cm.__exit__(None, None, None)
                wqkv_cm.__exit__(None, None, None)
                w2s_cm = tc.tile_pool(name="w2s", bufs=3)
                w2s = w2s_cm.__enter__()

            if STAGE >= 4:
                # ---- phase B: sequential behind the RS chunks ----
                def ffn2_pass(h):
                    """FFN2 for token tiles 2h, 2h+1: stream w2 in 256-row
                    chunks alternating two DMA rings, hT stationary, two
                    N=512 matmuls per k-chunk per tile, then LN2 + out."""
                    psA = pssc.tile([P, D], f32, tag="sc", name=f"psA_{h}")
                    psB = pssc.tile([P, D], f32, tag="sc", name=f"psB_{h}")
                    nk = F // (2 * P)
                    for c in range(nk):
                        w2c = w2s.tile(
                            [P, 2, D], bf16, tag="w2c", name=f"w2c_{h}_{c}"
                        )
                        ring = nc.sync if c % 2 == 0 else nc.scalar
                        ring.dma_start(
                            w2c,
                            w2[c * 2 * P : (c + 1) * 2 * P, :].rearrange(
                                "(kk p) n -> p kk n", p=P
                            ),
                        )
                        for kk in range(2):
                            k = 2 * c + kk
                            for t, ps in ((0, psA), (1, psB)):
                                csl = slice(t * P, (t + 1) * P)
                                for dh in range(2):
                                    nc.tensor.matmul(
                                        ps[:, dh * QT : (dh + 1) * QT],
                                        hT[:, k, csl],
                                        w2c[:, kk, dh * QT : (dh + 1) * QT],
                                        start=(k == 0),
                                        stop=(k == 2 * nk - 1),
                                    )
                    ln2_out(2 * h + 0, psA)
                    ln2_out(2 * h + 1, psB)

                ln1_vec(2)
                ln1_tp(2, range(D // P), psbig)
                # pass(0) must consume tiles {0,1} of hT before the tile-2
                # chunks overwrite their columns (hT holds 2 tiles at once)
                ffn2_pass(0)
                for ft in range(F // P):
                    ffn1_chunk(ft, 2, 1, psbig, relu_scalar=True)
                ln1_vec(3)
                ln1_tp(3, range(D // P), psbig)
                for ft in range(F // P):
                    ffn1_chunk(ft, 3, 1, psbig, relu_scalar=True)
                ffn2_pass(1)
            else:
                dummy = ln1p.tile([P, D], f32, tag="tmp", name="dummy")
                nc.vector.memset(dummy, 0.0)
                for st in range(SL // P):
                    nc.sync.dma_start(out[st * P : (st + 1) * P, :], dummy)

            av_cm.__exit__(None, None, None)
            sc_cm.__exit__(None, None, None)
            big_cm.__exit__(None, None, None)
            w2s_cm.__exit__(None, None, None)
            ln1p_cm.__exit__(None, None, None)
            zp_cm.__exit__(None, None, None)
            projp_cm.__exit__(None, None, None)
            normp_cm.__exit__(None, None, None)
            smallp_cm.__exit__(None, None, None)
            atp_cm.__exit__(None, None, None)
            keep_cm.__exit__(None, None, None)

    nc.compile()
    return nc


_NC_CACHE = []


def _get_nc():
    if not _NC_CACHE:
        _NC_CACHE.append(build_bass())
    return _NC_CACHE[0]


def _token_blocks(r):
    """Global token rows (within a batch element) owned by rank r, as NCK
    blocks of 128: block ck covers rows [512*ck + 128*r, 512*ck + 128*r + 128)."""
    return [slice(QT * ck + P * r, QT * ck + P * r + P) for ck in range(NCK)]


def make_in_maps(x, wq, wk, wv, w_proj, b_proj, w1, b1, w2, b2, g1, be1, g2, be2):
    x = np.asarray(x, dtype=np.float32)
    bp = np.asarray(b_proj, dtype=np.float32)
    cat = lambda w, h0: np.ascontiguousarray(
        np.concatenate(
            [np.asarray(w[h0 + i], dtype=np.float32) for i in range(HPC)], axis=1
        )
    ).astype(bfnp)
    cvec_rows = [b_proj, g1, be1, g2, be2, b2]
    cvec = np.ascontiguousarray(
        np.stack([np.asarray(v, dtype=np.float32) for v in cvec_rows])
    )
    w1c = np.ascontiguousarray(np.asarray(w1, dtype=np.float32)).astype(bfnp)
    w2c = np.ascontiguousarray(np.asarray(w2, dtype=np.float32)).astype(bfnp)
    b1c = np.ascontiguousarray(np.asarray(b1, dtype=np.float32))
    wpc = np.asarray(w_proj, dtype=np.float32)
    xTs = [np.ascontiguousarray(x[g].T.astype(bfnp)) for g in range(B)]
    in_maps = []
    for c in range(NCORES):
        g, r = divmod(c, TP)
        h0 = HPC * r
        # b_proj is folded into the residual here instead of on device
        xs_blocks = np.concatenate(
            [x[g, blk] + bp for blk in _token_blocks(r)], axis=0
        )
        in_maps.append(
            {
                "xT": xTs[g],
                "xs": np.ascontiguousarray(xs_blocks),
                "wq2": cat(wq, h0),
                "wk2": cat(wk, h0),
                "wv4": cat(wv, h0),
                "wp": np.ascontiguousarray(
                    wpc[HPC * HS * r : HPC * HS * (r + 1)].astype(bfnp)
                ),
                "w1": w1c,
                "w2": w2c,
                "cvec": cvec,
                "b1d": b1c,
            }
        )
    return in_maps


def assemble(results):
    full = np.empty((B, S, D), dtype=np.float32)
    for c in range(NCORES):
        g, r = divmod(c, TP)
        o = results[c]["out"]
        for ck, blk in enumerate(_token_blocks(r)):
            full[g, blk] = o[ck * P : (ck + 1) * P]
    return full


def kernel(**inputs):
    nc = _get_nc()
    in_maps = make_in_maps(**inputs)
    res = run_bass_kernel_spmd(nc, in_maps, core_ids=list(range(NCORES)))
    return assemble(res.results)


# revision 59
# speedup vs baseline: 1.0788x; 1.0003x over previous
"""Trainium2 Bass kernel for nn_Block_77318001263203 (dense transformer block).

Distribution over 8 NeuronCores: data-parallel over batch (2 groups of 4
cores) x tensor-parallel over heads (4 heads/core) for attention+proj,
4-way-chunked bf16 ReduceScatter of the proj partials over each 4-core
group (each chunk hands every rank one 128-token block, so rank r owns
the strided token set {512*ck + 128*r + j}), then token-parallel FFN with
replicated FFN weights — no second collective.

Schedule (v8), built for PE clock density — the HAM activity monitor
halves the PE clock whenever its duty cycle sags, and the v2 baseline
spent ~40% of its span at 1.2 GHz:
  - PE warm-up transposes + a dummy ReduceScatter at kernel start release
    the HAM clock gate and absorb the CC stream's first-trigger latency.
  - Startup DMAs spread over three rings in quarter-tile slices so the
    first QKV matmuls start on partially-landed weights; the broadcast
    const loads are deferred behind the latency-critical traffic.
  - QKV token tiles and attention row blocks interleave (tt0, tt1, qt0,
    tt2, qt1, tt3, qt2, qt3) so the scalar-engine exp stream overlaps
    QKV matmuls.
  - Attention processes key blocks in 2-block waves with one fused exp
    per wave (N=1024 activations) and AV matmuls trailing one wave.
  - A filler queue splices phase-B work (LN1 vector chains, x1T
    transposes, FFN1 chunks) between attention waves, several waves after
    each ReduceScatter chunk lands so nothing head-of-line-blocks the PE
    or vector queues on a collective.
  - FFN1 runs as 128/256-wide chunks against resident w1; FFN2 streams w2
    once per token-tile pair in 256-row chunks alternating two DMA rings,
    hT stationary, N=512 moving, both tiles accumulating per chunk.
  - PSUM pools are never closed mid-kernel (phase B reuses the attention
    pools) to avoid drain bubbles; only the QKV SBUF pools hand their
    space to the w2 stream, in attention(3)'s shadow.
  - Tail after RS3 lands is just tile 3's ln1 -> ffn1 -> ffn2 -> LN2
    chain, with the pair's two PSUM drains split across scalar + vector.

All matmuls bf16 (fp32 PSUM accumulate); b_proj is folded into the xs
residual host-side. Known HW faults respected from v2: no bf16 matmul
operands at partition offset 64, no f32r tile_position packing.

kernel(**inputs) takes the FULL inputs from setup_inputs() and returns the
FULL [2, 2048, 1024] float32 output.
"""

import numpy as np
import ml_dtypes

import concourse.bass as bass
import concourse.mybir as mybir
import concourse.tile as tile
from concourse import bacc
from concourse.bass_utils import run_bass_kernel_spmd
from concourse.masks import make_identity

# problem dims (hardcoded per the harness contract)
B, S, D = 2, 2048, 1024
H, HS, F = 16, 64, 4096
EPS = 1e-5
P = 128
NCORES = 8
TP = 4  # cores per batch group
HPC = H // TP  # heads per core = 4
SL = S // TP  # tokens owned per core = 512 (4 strided blocks of 128)
QT = 512  # query row tile (attention row granularity)
SUB = 256  # score/exp subtile width
KB = 128  # key block
NCK = 4  # reduce-scatter chunks
NEG = -1.0e9  # additive causal mask (exp underflows to exactly 0)

f32 = mybir.dt.float32
bf16 = mybir.dt.bfloat16
bfnp = ml_dtypes.bfloat16

REPLICA_GROUPS = [[0, 1, 2, 3], [4, 5, 6, 7]]


def _bcast_row_ap(t, row, width):
    """DMA-source AP broadcasting row `row` of DRAM tensor t to 128 partitions."""
    return bass.AP(tensor=t, offset=row * width, ap=[[0, P], [1, width]])


def build_bass():
    import os

    # debug bisection: 1=QKV, 2=+attention rows, 3=+proj/RS, 4=full
    STAGE = int(os.environ.get("KSTAGE", "4"))
    nc = bacc.Bacc("TRN2", target_bir_lowering=False, debug=False, num_devices=NCORES)

    xT = nc.dram_tensor("xT", [D, S], bf16, kind="ExternalInput").ap()
    xs = nc.dram_tensor("xs", [SL, D], f32, kind="ExternalInput").ap()
    wq2 = nc.dram_tensor("wq2", [D, HPC * HS], bf16, kind="ExternalInput").ap()
    wk2 = nc.dram_tensor("wk2", [D, HPC * HS], bf16, kind="ExternalInput").ap()
    wv4 = nc.dram_tensor("wv4", [D, HPC * HS], bf16, kind="ExternalInput").ap()
    wp = nc.dram_tensor("wp", [HPC * HS, D], bf16, kind="ExternalInput").ap()
    w1 = nc.dram_tensor("w1", [D, F], bf16, kind="ExternalInput").ap()
    w2 = nc.dram_tensor("w2", [F, D], bf16, kind="ExternalInput").ap()
    cvec = nc.dram_tensor("cvec", [6, D], f32, kind="ExternalInput").ap()
    b1d = nc.dram_tensor("b1d", [F], f32, kind="ExternalInput").ap()
    out = nc.dram_tensor("out", [SL, D], f32, kind="ExternalOutput").ap()

    # per-chunk collective bounce buffers (separate tensors -> precise deps)
    rs_in = [nc.dram_tensor(f"rs_in{c}", [QT, D], bf16) for c in range(NCK)]
    rs_out = [nc.dram_tensor(f"rs_out{c}", [P, D], bf16) for c in range(NCK)]
    # tiny dummy collective fired at kernel start: absorbs the CC stream's
    # first-trigger startup latency (~11us) so RS chunk 0 starts promptly
    cc_warm_in = nc.dram_tensor("cc_warm_in", [TP, 1], bf16)
    cc_warm_out = nc.dram_tensor("cc_warm_out", [1, 1], bf16)

    # additive causal mask [all-NEG block | lower-triangle-NEG block]:
    # mfull[:, KB:] is the triangle alone (keep 0 where key t <= query q).
    tri = np.where(
        np.arange(KB)[:, None] <= np.arange(KB)[None, :], 0.0, NEG
    ).astype(np.float32)
    full = np.concatenate([np.full((KB, KB), NEG, np.float32), tri], axis=1)
    m_full_dram = nc.inline_tensor(np.ascontiguousarray(full), name="mask_full")

    with tile.TileContext(nc) as tc:
        with tc.tile_pool(name="const", bufs=1) as constp:
            identb = constp.tile([P, P], bf16)
            make_identity(nc, identb)
            eps_t = constp.tile([P, 1], f32)
            nc.vector.memset(eps_t, EPS)
            # const tiles allocated here; their broadcast DMAs (which write
            # 128x the source bytes into SBUF) are deferred until after the
            # startup-critical QKV loads are in the rings
            b1_sb = constp.tile([P, F // P], f32)
            g1b = constp.tile([P, D], f32)
            be1b = constp.tile([P, D], f32)
            g2b = constp.tile([P, D], f32)
            be2b = constp.tile([P, D], f32)
            b2b = constp.tile([P, D], bf16)

            keep_cm = tc.tile_pool(name="keep", bufs=1)
            keep = keep_cm.__enter__()
            mfull_sb = keep.tile([P, 2 * KB], f32, tag="mfull")
            mtri_sb = mfull_sb[:, KB : 2 * KB]

            # wp's dma is emitted after wq below (same ring, wq is urgent)
            wp_sb = keep.tile([P, (HPC * HS) // P, D], bf16, tag="wp")

            # attention working set. q/k live on partitions 0-63 with the
            # head parity in the free dim: bf16 matmul operands fault on
            # HW at partition offset 64, so every score matmul reads
            # partition-offset-0 slices.
            q2T = keep.tile([HS, 2, 2, S], bf16, tag="q2T")
            k2T = keep.tile([HS, 2, 2, S], bf16, tag="k2T")
            v4e = keep.tile([P, S // P, HPC * (HS + 1)], bf16, tag="v4e")
            ones4 = keep.tile([P, HPC, 1], bf16, tag="ones4")
            nc.vector.memset(ones4, 1.0)

            # phase B persistents
            w1_sb = keep.tile([P, D // P, F], bf16, tag="w1")
            x1T = keep.tile([P, D // P, SL], bf16, tag="x1T")
            x1r = keep.tile([P, SL // P, D], bf16, tag="x1r")
            hT = keep.tile([P, F // P, SL // 2], bf16, tag="hT")

            # PE warm-up: ~48 back-to-back transposes (~5us of PE activity)
            # run during the initial weight DMAs so the HAM clock gate is
            # released before the first real matmul.
            with tc.tile_pool(name="ps_warm", bufs=1, space="PSUM") as pswarm:
                wtile = pswarm.tile([P, P], bf16, tag="warm")
                for _ in range(48):
                    nc.tensor.transpose(wtile, identb, identb)
            if STAGE >= 3:
                nc.gpsimd.collective_compute(
                    "ReduceScatter",
                    mybir.AluOpType.add,
                    replica_groups=REPLICA_GROUPS,
                    ins=[cc_warm_in.ap().opt()],
                    outs=[cc_warm_out.ap().opt()],
                )

            # ---------------- interleaved QKV + attention ----------------
            # SBUF pool stack: pools that outlive QKV open first so the
            # QKV pools (wqkv, xr) can close mid-kernel in LIFO order.
            atp_cm = tc.tile_pool(name="atp", bufs=1)
            atp = atp_cm.__enter__()
            smallp_cm = tc.tile_pool(name="smallp", bufs=2)
            smallp = smallp_cm.__enter__()
            normp_cm = tc.tile_pool(name="normp", bufs=2)
            normp = normp_cm.__enter__()
            projp_cm = tc.tile_pool(name="projp", bufs=1)
            projp = projp_cm.__enter__()
            zp_cm = tc.tile_pool(name="zp", bufs=2)
            zp = zp_cm.__enter__()
            ln1p_cm = tc.tile_pool(name="ln1p", bufs=1)
            ln1p = ln1p_cm.__enter__()

            # QKV weights in ko-half loads so the first accumulation chunks
            # can begin before the full weight tile lands
            wqkv_cm = tc.tile_pool(name="wqkv", bufs=1)
            wqkvp = wqkv_cm.__enter__()
            wq_sb = wqkvp.tile([P, D // P, HPC * HS], bf16, tag="wq")
            wk_sb = wqkvp.tile([P, D // P, HPC * HS], bf16, tag="wk")
            wv_sb = wqkvp.tile([P, D // P, HPC * HS], bf16, tag="wv")
            for h in range(4):
                hsl = slice(h * 2, (h + 1) * 2)
                rsl = slice(h * (D // 4), (h + 1) * (D // 4))
                nc.scalar.dma_start(
                    wq_sb[:, hsl, :],
                    wq2[rsl, :].rearrange("(ko p) m -> p ko m", p=P),
                )
                nc.gpsimd.dma_start(
                    wk_sb[:, hsl, :],
                    wk2[rsl, :].rearrange("(ko p) m -> p ko m", p=P),
                )
                nc.gpsimd.dma_start(
                    wv_sb[:, hsl, :],
                    wv4[rsl, :].rearrange("(ko p) m -> p ko m", p=P),
                )
            nc.scalar.dma_start(wp_sb, wp.rearrange("(ko p) n -> p ko n", p=P))

            def deferred_consts():
                """Const loads emitted behind the urgent QKV ring traffic."""
                nc.sync.dma_start(mfull_sb, m_full_dram.ap())
                nc.sync.dma_start(b1_sb, b1d.rearrange("(ko p) -> p ko", p=P))
                nc.sync.dma_start(g1b, _bcast_row_ap(cvec.tensor, 1, D))
                nc.sync.dma_start(be1b, _bcast_row_ap(cvec.tensor, 2, D))
                nc.sync.dma_start(g2b, _bcast_row_ap(cvec.tensor, 3, D))
                nc.sync.dma_start(be2b, _bcast_row_ap(cvec.tensor, 4, D))

            xr_cm = tc.tile_pool(name="xrp", bufs=1)
            xrp = xr_cm.__enter__()
            big_cm = tc.tile_pool(name="ps_big", bufs=2, space="PSUM")
            psbig = big_cm.__enter__()
            sc_cm = tc.tile_pool(name="ps_sc", bufs=2, space="PSUM")
            pssc = sc_cm.__enter__()
            av_cm = tc.tile_pool(name="ps_av", bufs=2, space="PSUM")
            psav = av_cm.__enter__()

            def qkv_tile(tt):
                xr = xrp.tile([P, D // P, QT], bf16, tag="xr", name=f"xr_{tt}")
                # quarter-loads so the first ko chunks' matmuls can start
                # before the whole tile has landed
                for h in range(4):
                    nc.sync.dma_start(
                        xr[:, h * 2 : (h + 1) * 2, :],
                        xT[h * (D // 4) : (h + 1) * (D // 4),
                           tt * QT : (tt + 1) * QT].rearrange(
                            "(ko p) m -> p ko m", p=P
                        ),
                    )
                for hp in range(2):
                    qps = psbig.tile([P, QT], f32, tag="big", name=f"qps_{tt}_{hp}")
                    for ko in range(D // P):
                        nc.tensor.matmul(
                            qps,
                            wq_sb[:, ko, hp * P : (hp + 1) * P],
                            xr[:, ko, :],
                            start=(ko == 0),
                            stop=(ko == D // P - 1),
                        )
                    for par in range(2):
                        nc.vector.tensor_copy(
                            q2T[:, par, hp, tt * QT : (tt + 1) * QT],
                            qps[par * HS : (par + 1) * HS, :],
                        )
                    kps = psbig.tile([P, QT], f32, tag="big", name=f"kps_{tt}_{hp}")
                    for ko in range(D // P):
                        nc.tensor.matmul(
                            kps,
                            wk_sb[:, ko, hp * P : (hp + 1) * P],
                            xr[:, ko, :],
                            start=(ko == 0),
                            stop=(ko == D // P - 1),
                        )
                    for par in range(2):
                        nc.vector.tensor_copy(
                            k2T[:, par, hp, tt * QT : (tt + 1) * QT],
                            kps[par * HS : (par + 1) * HS, :],
                        )
                for mt in range(QT // P):
                    vps = psbig.tile(
                        [P, HPC * HS], f32, tag="big",
                        padded_shape=[P, QT], name=f"vps_{tt}_{mt}",
                    )
                    for ko in range(D // P):
                        nc.tensor.matmul(
                            vps,
                            xr[:, ko, mt * P : (mt + 1) * P],
                            wv_sb[:, ko, :],
                            start=(ko == 0),
                            stop=(ko == D // P - 1),
                        )
                    idx = tt * (QT // P) + mt
                    vv = v4e[:, idx, :].rearrange("p (h e) -> p h e", e=HS + 1)
                    nc.vector.tensor_copy(
                        vv[:, :, 0:HS],
                        vps.rearrange("p (h e) -> p h e", e=HS),
                    )
                    nc.vector.tensor_copy(vv[:, :, HS : HS + 1], ones4)
                # the big w1 load rides behind the latency-critical QKV
                # loads in 2MB chunks (needed only from attention(3) on)
                nc.scalar.dma_start(
                    w1_sb[:, :, tt * (F // 4) : (tt + 1) * (F // 4)],
                    w1[:, tt * (F // 4) : (tt + 1) * (F // 4)].rearrange(
                        "(ko p) m -> p ko m", p=P
                    ),
                )

            # PE filler work items (thunks) spliced between attention waves:
            # keeps the PE duty cycle high through the scalar-bound exp
            # stream so the HAM clock gate stays released, and moves phase-B
            # matmuls off the critical tail.
            fillers = []

            def attention(qt, fill_from=10 ** 9):
                """Score/exp/AV waves + normalize for query rows
                [qt*QT, (qt+1)*QT), then proj + RS chunk qt."""
                nkb = 4 * qt + 4
                nwaves = 0
                attnT = atp.tile([P, 2, QT], bf16, tag="attnT", name=f"attnT_{qt}")
                for hp in range(2):
                    apair = psav.tile([HS + 1, QT], f32, tag="at", name=f"ap_{qt}_{hp}")
                    apodd = psav.tile([HS + 1, QT], f32, tag="at", name=f"ao_{qt}_{hp}")
                    for sub in range(QT // SUB):
                        qlo = qt * QT + sub * SUB
                        live = [kb for kb in range(nkb) if KB * kb < qlo + SUB]
                        n = len(live)
                        ssl = slice(sub * SUB, (sub + 1) * SUB)
                        waves = [live[i : i + 2] for i in range(0, n, 2)]
                        prev = None  # (ee, wave, i0)

                        def emit_av(ee, wave, i0):
                            for b, kb in enumerate(wave):
                                i = i0 + b
                                for par, aps in ((0, apair), (1, apodd)):
                                    he = (2 * hp + par) * (HS + 1)
                                    nc.tensor.matmul(
                                        aps[:, ssl],
                                        v4e[:, kb, he : he + HS + 1],
                                        ee[:, b, par, :],
                                        start=(i == 0),
                                        stop=(i == n - 1),
                                    )

                        i0 = 0
                        for w, wave in enumerate(waves):
                            nw = len(wave)
                            assert nw == 2, "live block counts are even"
                            sp = pssc.tile(
                                [P, 2, 2, SUB], f32, tag="sc",
                                name=f"sp_{qt}_{hp}_{sub}_{w}",
                            )
                            for b, kb in enumerate(wave):
                                ksl = slice(kb * KB, (kb + 1) * KB)
                                for par in range(2):
                                    nc.tensor.matmul(
                                        sp[:, b, par, :],
                                        k2T[:, par, hp, ksl],
                                        q2T[:, par, hp, qlo : qlo + SUB],
                                        start=True,
                                        stop=True,
                                    )
                                moff = KB * kb - qlo
                                if moff == 0:
                                    nc.vector.tensor_add(
                                        sp[:, b, :, 0:KB],
                                        sp[:, b, :, 0:KB],
                                        mtri_sb[:, None, :].to_broadcast((P, 2, KB)),
                                    )
                                elif moff == KB:
                                    nc.vector.tensor_add(
                                        sp[:, b, :, :],
                                        sp[:, b, :, :],
                                        mfull_sb[:, None, :].to_broadcast(
                                            (P, 2, SUB)
                                        ),
                                    )
                            if prev is not None:
                                emit_av(*prev)
                            ee = smallp.tile(
                                [P, 2, 2, SUB], bf16, tag="ee",
                                name=f"ee_{qt}_{hp}_{sub}_{w}",
                            )
                            nc.scalar.activation(
                                out=ee,
                                in_=sp,
                                func=mybir.ActivationFunctionType.Exp,
                                scale=float(HS) ** -0.5,
                            )
                            prev = (ee, wave, i0)
                            i0 += nw
                            nwaves += 1
                            # filler splicing starts several waves in so the
                            # filler's upstream deps (RS + LN1 vector chain)
                            # have landed and never stall the PE queue
                            if nwaves > fill_from and fillers:
                                fillers.pop(0)()
                        if prev is not None:
                            emit_av(*prev)
                    # normalize in SBUF off the PE critical path; both
                    # PSUM copies go first so the accumulator banks free
                    # up as fast as possible
                    uas = []
                    for par, aps in ((0, apair), (1, apodd)):
                        ua = normp.tile(
                            [HS + 1, QT], f32, tag="ua", name=f"ua_{qt}_{hp}_{par}"
                        )
                        nc.vector.tensor_copy(ua, aps)
                        uas.append(ua)
                    for par in range(2):
                        ua = uas[par]
                        rec = normp.tile([1, QT], f32, tag="rec", bufs=1)
                        nc.vector.reciprocal(rec, ua[HS : HS + 1, :])
                        bc = normp.tile([HS, QT], f32, tag="bc", bufs=1)
                        nc.gpsimd.partition_broadcast(bc, rec)
                        nc.vector.tensor_mul(
                            attnT[par * HS : (par + 1) * HS, hp, :],
                            ua[0:HS, :],
                            bc,
                        )
                if STAGE < 3:
                    return
                # proj for this qt's 4 token tiles, then RS chunk qt
                for mtl in range(4):
                    mt = 4 * qt + mtl
                    prj = projp.tile([P, D], bf16, tag="prj", name=f"prj_{mt}")
                    for nh in range(D // QT):
                        pps = psbig.tile(
                            [P, QT], f32, tag="big", name=f"pps_{mt}_{nh}"
                        )
                        for ko in range(2):
                            nc.tensor.matmul(
                                pps,
                                attnT[:, ko, mtl * P : (mtl + 1) * P],
                                wp_sb[:, ko, nh * QT : (nh + 1) * QT],
                                start=(ko == 0),
                                stop=(ko == 1),
                            )
                        nc.vector.tensor_copy(prj[:, nh * QT : (nh + 1) * QT], pps)
                    nc.sync.dma_start(
                        rs_in[qt].ap()[mtl * P : (mtl + 1) * P, :], prj
                    )
                nc.gpsimd.collective_compute(
                    "ReduceScatter",
                    mybir.AluOpType.add,
                    replica_groups=REPLICA_GROUPS,
                    ins=[rs_in[qt].ap().opt()],
                    outs=[rs_out[qt].ap().opt()],
                )

            # phase B pools (SBUF side; PSUM scratch opened after attention)
            # b2 broadcast lands in a transient f32 tile, kept only as bf16
            b2f = ln1p.tile([P, D], f32, tag="tmp", name="b2f")
            nc.gpsimd.dma_start(b2f, _bcast_row_ap(cvec.tensor, 5, D))
            nc.vector.tensor_copy(b2b, b2f)

            def ln1_vec(st):
                """rs_out[st] + xs[st] (b_proj pre-folded) -> LN1 -> x1r
                (bf16). Vector-only: the x1T transposes are emitted
                separately (ln1_tp) so they never head-of-line-block the
                PE queue behind a ReduceScatter."""
                yb = zp.tile([P, D], bf16, tag="z", name=f"yb_{st}")
                nc.gpsimd.dma_start(yb, rs_out[st].ap())
                y = ln1p.tile([P, D], f32, tag="y")
                nc.vector.tensor_copy(y, yb)
                xst = ln1p.tile([P, D], f32, tag="tmp", name=f"xst_{st}")
                nc.gpsimd.dma_start(xst, xs[st * P : (st + 1) * P, :])
                nc.vector.tensor_add(y, y, xst)
                stats = ln1p.tile([P, 2, 6], f32, tag="stats")
                yv = y.rearrange("p (s d) -> p s d", s=2)
                nc.vector.bn_stats(out=stats[:, 0, :], in_=yv[:, 0, :])
                nc.vector.bn_stats(out=stats[:, 1, :], in_=yv[:, 1, :])
                mv = ln1p.tile([P, 2], f32, tag="mv")
                nc.vector.bn_aggr(out=mv, in_=stats)
                rstd = ln1p.tile([P, 1], f32, tag="rstd")
                nc.scalar.activation(
                    out=rstd,
                    in_=mv[:, 1:2],
                    func=mybir.ActivationFunctionType.Sqrt,
                    bias=eps_t,
                    scale=1.0,
                )
                nc.vector.reciprocal(rstd, rstd)
                tmp = ln1p.tile([P, D], f32, tag="tmp")
                nc.vector.tensor_scalar(
                    out=tmp,
                    in0=y,
                    scalar1=mv[:, 0:1],
                    scalar2=rstd,
                    op0=mybir.AluOpType.subtract,
                    op1=mybir.AluOpType.mult,
                )
                nc.vector.tensor_mul(tmp, tmp, g1b)
                nc.vector.tensor_add(x1r[:, st, :], tmp, be1b)

            def ln1_tp(st, dks, tpool):
                """x1T transposes for token tile st, dk chunks `dks`."""
                for dk in dks:
                    tp = tpool.tile(
                        [P, P], bf16, tag="big", name=f"tp_{st}_{dk}",
                    )
                    nc.tensor.transpose(
                        tp, x1r[:, st, dk * P : (dk + 1) * P], identb
                    )
                    nc.vector.tensor_copy(x1T[:, dk, st * P : (st + 1) * P], tp)

            def ffn1_chunk(ft, st, nt, pool, relu_scalar=False):
                """FFN1 for the ft-th 128-wide hidden chunk over `nt` token
                tiles starting at tile st (N = nt*128); relu+bias on vector
                (filler mode) or scalar (phase B, where vector is loaded)."""
                in_sl = slice(st * P, (st + nt) * P)
                out_sl = slice((st % 2) * P, ((st % 2) + nt) * P)
                hps = pool.tile(
                    [P, nt * P], f32, tag="big",
                    padded_shape=[P, QT], name=f"hc_{ft}_{st}_{nt}",
                )
                for ko in range(D // P):
                    nc.tensor.matmul(
                        hps,
                        w1_sb[:, ko, ft * P : (ft + 1) * P],
                        x1T[:, ko, in_sl],
                        start=(ko == 0),
                        stop=(ko == D // P - 1),
                    )
                if relu_scalar:
                    nc.scalar.activation(
                        out=hT[:, ft, out_sl],
                        in_=hps,
                        func=mybir.ActivationFunctionType.Relu,
                        bias=b1_sb[:, ft : ft + 1],
                        scale=1.0,
                    )
                else:
                    nc.vector.tensor_scalar(
                        out=hT[:, ft, out_sl],
                        in0=hps,
                        scalar1=b1_sb[:, ft : ft + 1],
                        scalar2=0.0,
                        op0=mybir.AluOpType.add,
                        op1=mybir.AluOpType.max,
                    )

            def ln2_out(st, ps, eng=None):
                """z = ps + x1r[st] + b2 -> LN2 -> out for token tile st.
                Odd tiles drain PSUM via the scalar engine so the two
                drains of a pass run in parallel."""
                eng = nc.vector
                zm = zp.tile([P, D], bf16, tag="z", name=f"z_{st}")
                if st % 2 == 1:
                    nc.scalar.copy(zm, ps)
                else:
                    eng.tensor_copy(zm, ps)
                eng.tensor_add(zm, zm, x1r[:, st, :])
                eng.tensor_add(zm, zm, b2b)
                stats = ln1p.tile([P, 2, 6], f32, tag="stats")
                zv = zm.rearrange("p (s d) -> p s d", s=2)
                nc.vector.bn_stats(out=stats[:, 0, :], in_=zv[:, 0, :])
                nc.vector.bn_stats(out=stats[:, 1, :], in_=zv[:, 1, :])
                mv = ln1p.tile([P, 2], f32, tag="mv")
                nc.vector.bn_aggr(out=mv, in_=stats)
                rstd = ln1p.tile([P, 1], f32, tag="rstd")
                nc.scalar.activation(
                    out=rstd,
                    in_=mv[:, 1:2],
                    func=mybir.ActivationFunctionType.Sqrt,
                    bias=eps_t,
                    scale=1.0,
                )
                nc.vector.reciprocal(rstd, rstd)
                o = ln1p.tile([P, D], f32, tag="tmp", name=f"o_{st}")
                eng.tensor_scalar(
                    out=o,
                    in0=zm,
                    scalar1=mv[:, 0:1],
                    scalar2=rstd,
                    op0=mybir.AluOpType.subtract,
                    op1=mybir.AluOpType.mult,
                )
                eng.tensor_mul(o, o, g2b)
                eng.tensor_add(o, o, be2b)
                nc.sync.dma_start(out[st * P : (st + 1) * P, :], o)

            # ---- emission: QKV tiles interleaved with attention rows ----
            qkv_tile(0)
            deferred_consts()
            qkv_tile(1)
            if STAGE >= 2:
                attention(0)
            qkv_tile(2)
            if STAGE >= 2:
                attention(1)
                if STAGE >= 4:
                    # RS0 lands mid-attention(2). Even the LN1 vector chain
                    # is a late-spliced filler: emitted early it would
                    # head-of-line-block the vector queue (mask adds) on
                    # the RS0-gated rs_out read. Tile-0-only FFN1 chunks
                    # follow as PE density filler; LN1 of tile 1 splices in
                    # last (RS1 lands mid-attention(2)).
                    fillers.append(lambda: ln1_vec(0))
                    fillers.extend(
                        [lambda a=dk: ln1_tp(0, [a, a + 1], psbig)
                         for dk in range(0, D // P, 2)]
                    )
                    fillers.extend(
                        [lambda a=ft: ffn1_chunk(a, 0, 1, psbig)
                         for ft in range(8)]
                    )
                    fillers.append(lambda: ln1_vec(1))
                    fillers.extend(
                        [lambda a=dk: ln1_tp(1, [a, a + 1], psbig)
                         for dk in range(0, D // P, 2)]
                    )
            qkv_tile(3)
            if STAGE >= 2:
                attention(2, fill_from=4)
                if STAGE >= 4:
                    fillers.extend(
                        [lambda a=ft: ffn1_chunk(a, 1, 1, psbig)
                         for ft in range(8)]
                    )
                    fillers.extend(
                        [lambda a=ft: ffn1_chunk(a, 0, 2, psbig)
                         for ft in range(8, F // P)]
                    )

                # QKV SBUF pools close here, in attention(3)'s shadow; the
                # w2 streaming pool reuses their space. PSUM pools stay
                # open through phase B (psbig scratch, pssc FFN2 accum) so
                # no PSUM drain bubbles are inserted.
                xr_cm.__exit__(None, None, None)
                wqkv_cm.__exit__(None, None, None)
                w2s_cm = tc.tile_pool(name="w2s", bufs=4)
                w2s = w2s_cm.__enter__()

                attention(3, fill_from=4)
                # flush any filler items the attention waves didn't absorb
                while fillers:
                    fillers.pop(0)()
            else:
                xr_cm.__exit__(None, None, None)
                wqkv_cm.__exit__(None, None, None)
                w2s_cm = tc.tile_pool(name="w2s", bufs=3)
                w2s = w2s_cm.__enter__()

            if STAGE >= 4:
                # ---- phase B: sequential behind the RS chunks ----
                def ffn2_pass(h):
                    """FFN2 for token tiles 2h, 2h+1: stream w2 in 256-row
                    chunks alternating two DMA rings, hT stationary, two
                    N=512 matmuls per k-chunk per tile, then LN2 + out."""
                    psA = pssc.tile([P, D], f32, tag="sc", name=f"psA_{h}")
                    psB = pssc.tile([P, D], f32, tag="sc", name=f"psB_{h}")
                    nk = F // (2 * P)
                    for c in range(nk):
                        w2c = w2s.tile(
                            [P, 2, D], bf16, tag="w2c", name=f"w2c_{h}_{c}"
                        )
                        ring = nc.sync if c % 2 == 0 else nc.scalar
                        ring.dma_start(
                            w2c,
                            w2[c * 2 * P : (c + 1) * 2 * P, :].rearrange(
                                "(kk p) n -> p kk n", p=P
                            ),
                        )
                        for kk in range(2):
                            k = 2 * c + kk
                            for t, ps in ((0, psA), (1, psB)):
                                csl = slice(t * P, (t + 1) * P)
                                for dh in range(2):
                                    nc.tensor.matmul(
                                        ps[:, dh * QT : (dh + 1) * QT],
                                        hT[:, k, csl],
                                        w2c[:, kk, dh * QT : (dh + 1) * QT],
                                        start=(k == 0),
                                        stop=(k == 2 * nk - 1),
                                    )
                    ln2_out(2 * h + 0, psA)
                    ln2_out(2 * h + 1, psB)

                ln1_vec(2)
                ln1_tp(2, range(D // P), psbig)
                # pass(0) must consume tiles {0,1} of hT before the tile-2
                # chunks overwrite their columns (hT holds 2 tiles at once)
                ffn2_pass(0)
                for ft in range(F // P):
                    ffn1_chunk(ft, 2, 1, psbig, relu_scalar=True)
                ln1_vec(3)
                ln1_tp(3, range(D // P), psbig)
                for ft in range(F // P):
                    ffn1_chunk(ft, 3, 1, psbig, relu_scalar=True)
                ffn2_pass(1)
            else:
                dummy = ln1p.tile([P, D], f32, tag="tmp", name="dummy")
                nc.vector.memset(dummy, 0.0)
                for st in range(SL // P):
                    nc.sync.dma_start(out[st * P : (st + 1) * P, :], dummy)

            av_cm.__exit__(None, None, None)
            sc_cm.__exit__(None, None, None)
            big_cm.__exit__(None, None, None)
            w2s_cm.__exit__(None, None, None)
            ln1p_cm.__exit__(None, None, None)
            zp_cm.__exit__(None, None, None)
            projp_cm.__exit__(None, None, None)
            normp_cm.__exit__(None, None, None)
            smallp_cm.__exit__(None, None, None)
            atp_cm.__exit__(None, None, None)
            keep_cm.__exit__(None, None, None)

    nc.compile()
    return nc


_NC_CACHE = []


def _get_nc():
    if not _NC_CACHE:
        _NC_CACHE.append(build_bass())
    return _NC_CACHE[0]


def _token_blocks(r):
    """Global token rows (within a batch element) owned by rank r, as NCK
    blocks of 128: block ck covers rows [512*ck + 128*r, 512*ck + 128*r + 128)."""
    return [slice(QT * ck + P * r, QT * ck + P * r + P) for ck in range(NCK)]


def make_in_maps(x, wq, wk, wv, w_proj, b_proj, w1, b1, w2, b2, g1, be1, g2, be2):
    x = np.asarray(x, dtype=np.float32)
    bp = np.asarray(b_proj, dtype=np.float32)
    cat = lambda w, h0: np.ascontiguousarray(
        np.concatenate(
            [np.asarray(w[h0 + i], dtype=np.float32) for i in range(HPC)], axis=1
        )
    ).astype(bfnp)
    cvec_rows = [b_proj, g1, be1, g2, be2, b2]
    cvec = np.ascontiguousarray(
        np.stack([np.asarray(v, dtype=np.float32) for v in cvec_rows])
    )
    w1c = np.ascontiguousarray(np.asarray(w1, dtype=np.float32)).astype(bfnp)
    w2c = np.ascontiguousarray(np.asarray(w2, dtype=np.float32)).astype(bfnp)
    b1c = np.ascontiguousarray(np.asarray(b1, dtype=np.float32))
    wpc = np.asarray(w_proj, dtype=np.float32)
    xTs = [np.ascontiguousarray(x[g].T.astype(bfnp)) for g in range(B)]
    in_maps = []
    for c in range(NCORES):
        g, r = divmod(c, TP)
        h0 = HPC * r
        # b_proj is folded into the residual here instead of on device
        xs_blocks = np.concatenate(
            [x[g, blk] + bp for blk in _token_blocks(r)], axis=0
        )
        in_maps.append(
            {
                "xT": xTs[g],
                "xs": np.ascontiguousarray(xs_blocks),
                "wq2": cat(wq, h0),
                "wk2": cat(wk, h0),
                "wv4": cat(wv, h0),
                "wp": np.ascontiguousarray(
                    wpc[HPC * HS * r : HPC * HS * (r + 1)].astype(bfnp)
                ),
                "w1": w1c,
                "w2": w2c,
                "cvec": cvec,
                "b1d": b1c,
            }
        )
    return in_maps


def assemble(results):
    full = np.empty((B, S, D), dtype=np.float32)
    for c in range(NCORES):
        g, r = divmod(c, TP)
        o = results[c]["out"]
        for ck, blk in enumerate(_token_blocks(r)):
            full[g, blk] = o[ck * P : (ck + 1) * P]
    return full


def kernel(**inputs):
    nc = _get_nc()
    in_maps = make_in_maps(**inputs)
    res = run_bass_kernel_spmd(nc, in_maps, core_ids=list(range(NCORES)))
    return assemble(res.results)
